# revision 1
# baseline (speedup 1.0000x reference)
"""YOLO-v1-style loss on 8 Trainium2 NeuronCores (Bass/Tile).

Data-parallel over batch: each core gets 2048 of 16384 batch elements
([2048,7,7,30] -> 128 partitions x 784 cells x 30 channels), computes
per-partition partial sums for the 5 loss terms on-device, host combines.

Inputs are converted to bf16 on the host: halves DMA traffic and enables
the DVE 2x perf mode on contiguous tensor_tensor ops. The resulting
relative error on each loss term is ~1e-5 (sums over millions of terms:
rounding noise averages out; the systematic bias is ~(p^2+t^2)*var(eps)).

Self-contained: hardcodes all shapes; only needs numpy + concourse (bass).
"""

import numpy as np
import ml_dtypes

import concourse.bass as bass
import concourse.bacc as bacc
import concourse.tile as tile
import concourse.mybir as mybir
from concourse.bass_utils import run_bass_kernel_spmd

f32 = mybir.dt.float32
bf16 = mybir.dt.bfloat16
Alu = mybir.AluOpType
Act = mybir.ActivationFunctionType
X = mybir.AxisListType.X

S = 7
B = 2
D = 30
BATCH = 16384
NCORES = 8
PER = BATCH // NCORES          # 2048 batch elems per core
P = 128                        # partitions
F = PER * S * S // P           # 784 cells per partition
NCHUNK = 4
CH = F // NCHUNK               # cells per partition per chunk
NACC = NCHUNK * 5              # accumulator columns (5 terms per chunk)

INV_S = 1.0 / S


def _bc_box(x):
    """[P, CH, ...] -> [P, 2, CH, ...]: broadcast over the box dim (step 0)."""
    return bass.AP(tensor=x.tensor, offset=x.offset,
                   ap=[x.ap[0], [0, 2]] + list(x.ap[1:]))


def _flat2(x, n):
    """Contiguous [P, 2, ch, 2] tile -> 2-free-dim view [[2, n], [1, 2]]."""
    return bass.AP(tensor=x.tensor, offset=x.offset,
                   ap=[x.ap[0], [2, n], [1, 2]])


def _bc_pair(x, n):
    """Contiguous [P, 2, ch] tile -> [[1, n], [0, 2]] (repeat each value 2x)."""
    return bass.AP(tensor=x.tensor, offset=x.offset,
                   ap=[x.ap[0], [1, n], [0, 2]])


def build_nc(f=F, nchunk=NCHUNK, repeat=1, variant="full"):
    ch = f // nchunk
    nacc = nchunk * 5
    nc = bacc.Bacc("TRN2", target_bir_lowering=False, debug=False,
                   num_devices=NCORES)
    # hybrid inputs, host-pre-split: box channels (0..9) f32 for exact
    # IoU/selection, class channels (10..29) bf16 (error averages out).
    box = nc.dram_tensor("box", [2, P, f, 10], f32, kind="ExternalInput")
    cls_ = nc.dram_tensor("cls", [2, P, f, 20], bf16, kind="ExternalInput")
    out = nc.dram_tensor("acc_out", [P, nacc], f32, kind="ExternalOutput")
    box_pm = box.ap().rearrange("two p f d -> p two f d")
    cls_pm = cls_.ap().rearrange("two p f d -> p two f d")

    V = nc.vector
    A = nc.scalar
    G = nc.gpsimd

    with tile.TileContext(nc) as tc:
        with (
            tc.tile_pool(name="inp", bufs=2) as inp,
            tc.tile_pool(name="wk2", bufs=2) as wk2,
            tc.tile_pool(name="wk1", bufs=1) as wk1,
            tc.tile_pool(name="one", bufs=1) as one,
        ):
            acc = one.tile([P, nacc], f32)
            V.memset(acc, 0.0)

            for k in range(nchunk * repeat):
                k = k % nchunk
                c0 = k * ch

                boxt = inp.tile([P, 2, ch, 10], f32, tag="boxt")
                nc.sync.dma_start(boxt, box_pm[:, :, c0:c0 + ch, :])
                clst = inp.tile([P, 2, ch, 20], bf16, tag="clst")
                nc.sync.dma_start(clst, cls_pm[:, :, c0:c0 + ch, :])

                # box-major views [P, 2, ch, 5]
                pb = boxt[:, 0].rearrange("p c (b k) -> p b c k", b=2)
                tb = boxt[:, 1].rearrange("p c (b k) -> p b c k", b=2)
                pxyr = pb[:, :, :, 0:2]
                pwhr = pb[:, :, :, 2:4]
                pcfr = pb[:, :, :, 4]      # [P,2,ch] conf ch 4,9
                twhr = tb[:, :, :, 2:4]
                tcfr = tb[:, :, :, 4]
                txyr = tb[:, :, :, 0:2]
                t4 = tb[:, 0, :, 4]        # [P,ch] obj mask (exactly 0/1)

                obj = wk1.tile([P, ch], f32, tag="obj")
                A.activation(obj, t4, Act.Copy)
                if variant in ("full", "dve"):
                    # ---- ACT extractions ----
                    pwh = wk2.tile([P, 2, ch, 2], f32, tag="pwh")   # 0.5*w, 0.5*h
                    A.activation(pwh, pwhr, Act.Copy, scale=0.5)
                    pxy = wk2.tile([P, 2, ch, 2], f32, tag="pxy")   # x/S, y/S
                    A.activation(pxy, pxyr, Act.Copy, scale=INV_S)
                    twh = wk2.tile([P, 2, ch, 2], f32, tag="twh")
                    A.activation(twh, twhr, Act.Copy, scale=0.5)
                    txy = wk1.tile([P, ch, 2], f32, tag="txy")      # t box0
                    A.activation(txy, tb[:, 0, :, 0:2], Act.Copy, scale=INV_S)
                    noobjm = wk1.tile([P, ch], f32, tag="noobjm")
                    A.activation(noobjm, t4, Act.Copy, scale=-1.0, bias=1.0)
                    pconf = wk1.tile([P, 2, ch], f32, tag="pconf")
                    A.activation(pconf, pcfr, Act.Copy)

                    # ---- diffs from raw inputs (strided reads, 1x) ----
                    dxy = wk2.tile([P, 2, ch, 2], f32, tag="dxy")
                    V.tensor_tensor(dxy, pxyr, txyr, op=Alu.subtract)
                    d2xy = wk2.tile([P, 2, ch, 2], f32, tag="d2xy")
                    A.square(d2xy, dxy)
                    swh = wk2.tile([P, 2, ch, 2], f32, tag="swh")   # pw + tw
                    V.tensor_tensor(swh, pwhr, twhr, op=Alu.add)
                    qwh = wk2.tile([P, 2, ch, 2], f32, tag="qwh")   # (pw/2)(tw/2)
                    V.tensor_tensor(qwh, pwh, twh, op=Alu.mult)
                    rwh = wk2.tile([P, 2, ch, 2], f32, tag="rwh")    # 2*sqrt(pw*tw)
                    A.activation(rwh, qwh, Act.Sqrt, scale=16.0)
                    dconf = wk1.tile([P, 2, ch], f32, tag="dconf")
                    V.tensor_tensor(dconf, pcfr, tcfr, op=Alu.subtract)
                    A.square(dconf, dconf)                  # in-place -> d2conf
                if variant in ("full", "pool"):
                    # class diffs (bf16, on POOL)
                    dcls = wk2.tile([P, ch, 20], bf16, tag="dcls", bufs=2)
                    G.tensor_tensor(dcls, clst[:, 0], clst[:, 1],
                                    op=Alu.subtract)
                    # mask by obj on POOL, square+accumulate on ACT
                    dm = wk2.tile([P, ch, 20], bf16, tag="dm", bufs=1)
                    objbc = bass.AP(tensor=obj.tensor, offset=obj.offset,
                                    ap=[obj.ap[0], [1, ch], [0, 20]])
                    G.tensor_tensor(dm, dcls, objbc, op=Alu.mult)

                if variant in ("full", "dve"):
                    # ---- corners (bf16 contiguous, 2x) ----
                    pc1 = wk2.tile([P, 2, ch, 2], f32, tag="pc1")
                    V.tensor_tensor(pc1, pxy, pwh, op=Alu.subtract)
                    pc2 = wk2.tile([P, 2, ch, 2], f32, tag="pc2")
                    V.tensor_tensor(pc2, pxy, pwh, op=Alu.add)
                    tc1 = wk1.tile([P, ch, 2], f32, tag="tc1")
                    V.tensor_tensor(tc1, txy, twh[:, 0], op=Alu.subtract)
                    tc2 = wk1.tile([P, ch, 2], f32, tag="tc2")
                    V.tensor_tensor(tc2, txy, twh[:, 0], op=Alu.add)

                    # ---- IoU ----
                    lt = wk1.tile([P, 2, ch, 2], f32, tag="lt")
                    V.tensor_tensor(lt, pc1, _bc_box(tc1), op=Alu.max)
                    rb = wk1.tile([P, 2, ch, 2], f32, tag="rb")
                    V.tensor_tensor(rb, pc2, _bc_box(tc2), op=Alu.min)
                    whd = wk1.tile([P, 2, ch, 2], f32, tag="whd")
                    V.tensor_tensor(whd, rb, lt, op=Alu.subtract)
                    A.activation(whd, whd, Act.Relu)        # in-place clamp >= 0
                    inter = wk1.tile([P, 2, ch], f32, tag="inter")
                    V.tensor_tensor(inter, whd[:, :, :, 0], whd[:, :, :, 1],
                                    op=Alu.mult)
                    areap = wk1.tile([P, 2, ch], f32, tag="areap")
                    V.tensor_tensor(areap, pb[:, :, :, 2], pb[:, :, :, 3],
                                    op=Alu.mult)
                    areat = wk1.tile([P, ch], f32, tag="areat")
                    V.tensor_tensor(areat, tb[:, 0, :, 2], tb[:, 0, :, 3],
                                    op=Alu.mult)
                    denom = wk1.tile([P, 2, ch], f32, tag="denom")
                    V.tensor_tensor(denom, areap, _bc_box(areat), op=Alu.add)
                    V.tensor_tensor(denom, denom, inter, op=Alu.subtract)
                    rden = wk1.tile([P, 2, ch], f32, tag="rden")
                    V.reciprocal_approx_fast(rden, denom)
                    iou = wk1.tile([P, 2, ch], f32, tag="iou")
                    V.tensor_tensor(iou, inter, rden, op=Alu.mult)

                    # ---- responsibility selection ----
                    ge = wk1.tile([P, ch], f32, tag="ge")
                    V.tensor_tensor(ge, iou[:, 0], iou[:, 1], op=Alu.is_ge)
                    miou = wk1.tile([P, ch], f32, tag="miou")
                    V.tensor_tensor(miou, iou[:, 0], iou[:, 1], op=Alu.max)
                    resp = wk1.tile([P, 2, ch], f32, tag="resp")
                    V.tensor_tensor(resp[:, 0], ge, obj, op=Alu.mult)
                    V.tensor_tensor(resp[:, 1], obj, resp[:, 0], op=Alu.subtract)

                # ---- loss terms -> acc columns (stt with fused accum) ----
                a0 = k * 5
                n2 = 2 * ch
                if variant in ("full", "dve"):
                    scr = wk1.tile([P, 2, ch, 2], f32, tag="scr", bufs=2)
                    V.scalar_tensor_tensor(
                        _flat2(scr, n2), _flat2(d2xy, n2), 0.0, _bc_pair(resp, n2),
                        op0=Alu.bypass, op1=Alu.mult,
                        accum_out=acc[:, a0 + 0:a0 + 1])

                    term = wk1.tile([P, 2, ch, 2], f32, tag="scr", bufs=2)
                    V.tensor_tensor(term, swh, rwh, op=Alu.subtract)
                    scr = wk1.tile([P, 2, ch, 2], f32, tag="scr", bufs=2)
                    V.scalar_tensor_tensor(
                        _flat2(scr, n2), _flat2(term, n2), 0.0, _bc_pair(resp, n2),
                        op0=Alu.bypass, op1=Alu.mult,
                        accum_out=acc[:, a0 + 1:a0 + 2])

                    odiff = wk1.tile([P, 2, ch], f32, tag="odiff")
                    V.tensor_tensor(odiff, pconf, _bc_box(miou), op=Alu.subtract)
                    osq = wk1.tile([P, 2, ch], f32, tag="osq")
                    A.square(osq, odiff)
                    scr = wk1.tile([P, 2, ch, 2], f32, tag="scr", bufs=2)
                    V.scalar_tensor_tensor(
                        scr[:, :, :, 0], osq, 0.0, resp,
                        op0=Alu.bypass, op1=Alu.mult,
                        accum_out=acc[:, a0 + 2:a0 + 3])

                    nb = wk1.tile([P, ch], f32, tag="nb")
                    V.tensor_tensor(nb, dconf[:, 0], dconf[:, 1], op=Alu.add)
                    scr = wk1.tile([P, 2, ch, 2], f32, tag="scr", bufs=2)
                    V.scalar_tensor_tensor(
                        scr[:, 0, :, 0], nb, 0.0, noobjm,
                        op0=Alu.bypass, op1=Alu.mult,
                        accum_out=acc[:, a0 + 3:a0 + 4])

                if variant in ("full", "pool"):
                    # class loss: sum((obj*d)^2) fused on ACT
                    A.activation(dm, dm, Act.Square,
                                 accum_out=acc[:, a0 + 4:a0 + 5])

            nc.sync.dma_start(out.ap(), acc)

    nc.compile()
    return nc


_NC_CACHE = None


def _get_nc():
    global _NC_CACHE
    if _NC_CACHE is None:
        _NC_CACHE = build_nc()
    return _NC_CACHE


def shard_inputs(pred_tensor, target_tensor):
    """Full [16384,7,7,30] f32 inputs -> per-core hybrid box(f32)/cls(bf16)."""
    p = np.ascontiguousarray(pred_tensor, dtype=np.float32).reshape(NCORES, P, F, D)
    t = np.ascontiguousarray(target_tensor, dtype=np.float32).reshape(NCORES, P, F, D)
    box = np.empty((NCORES, 2, P, F, 10), dtype=np.float32)
    box[:, 0] = p[..., 0:10]
    box[:, 1] = t[..., 0:10]
    cls_ = np.empty((NCORES, 2, P, F, 20), dtype=ml_dtypes.bfloat16)
    cls_[:, 0] = p[..., 10:30]
    cls_[:, 1] = t[..., 10:30]
    return [{"box": box[c], "cls": cls_[c]} for c in range(NCORES)]


def combine(results):
    """Per-core acc_out [P, NACC] -> 5-tuple of loss scalars."""
    total = np.zeros(5, dtype=np.float64)
    for r in results:
        a = r["acc_out"].astype(np.float64).sum(axis=0)   # [NACC]
        total += a.reshape(NCHUNK, 5).sum(axis=0)
    total /= BATCH
    return tuple(np.float32(v) for v in total)


def kernel(pred_tensor, target_tensor):
    nc = _get_nc()
    in_maps = shard_inputs(pred_tensor, target_tensor)
    res = run_bass_kernel_spmd(nc, in_maps, core_ids=list(range(NCORES)))
    return combine(res.results)



# revision 21
# speedup vs baseline: 1.6805x; 1.6805x over previous
"""YOLO-v1-style loss on 8 Trainium2 NeuronCores (Bass/Tile), v3.

Data-parallel over batch: each core gets 2048 of 16384 batch elements
(100,352 cells as 128 partitions x 784 cells); per-partition partial sums
for the 5 loss terms are combined on the host.

Layout: host repacks channels into per-channel planes ([P, plane, cells])
so DVE tensor_tensor ops run dense step-1 bf16 at 2x mode and
tensor_scalar ops at 4x. scalar_tensor_tensor (1x only) is avoided.

IoU via the overlap identity (no corner materialization):
  overlap_x = min(3.5*(pw+tw) - |px-tx|, 7*min(pw,tw)), clamped at 0.

Engine split:
  - DVE: box pipeline + responsibility masks + 8 class channels
  - ACT: sqrt, f32 conversions for reciprocal, square+accumulate passes
  - Pool: 12 class channels (cell-major tile: broadcast mask has its
    step-0 dim innermost - the only fast Q7 pattern) + SWDGE DMA descr gen
  - DMA CCE: class diff (p - t) computed inline: p streams in with an
    fp8->bf16 cast, host-negated t accumulates with cce add.

Explicit add_dep_helper edges order the CCE chain against its readers
(Tile's shadow tracking under-waits on multi-DMA read-modify-write tiles).

Self-contained: hardcodes all shapes; needs numpy + ml_dtypes + concourse.
"""

import numpy as np
import ml_dtypes

import concourse.bass as bass
import concourse.bacc as bacc
import concourse.tile as tile
import concourse.mybir as mybir
from concourse.bass_utils import run_bass_kernel_spmd
from bass_rust import add_dep_helper

DISABLE_CLS = False
DISABLE_CLS_G = False

f32 = mybir.dt.float32
bf16 = mybir.dt.bfloat16
f8e3 = mybir.dt.float8e3
Alu = mybir.AluOpType
Act = mybir.ActivationFunctionType

S = 7
BATCH = 16384
NCORES = 8
PER = BATCH // NCORES          # 2048 batch elems per core
P = 128                        # partitions
F = PER * S * S // P           # 784 cells per partition
NCQ = 4                        # class cell chunks
MQ = F // NCQ                  # 392
CHV = 8                        # class channels masked on DVE (channel-major)
CHG = 20 - CHV                 # class channels masked on Pool (cell-major)
HS = S / 2.0                   # 3.5
S2 = float(S * S)              # 49

NACC = 4 + 2 * NCQ


def _bc(x, r):
    """[P, ...] -> [P, r, ...]: broadcast (step-0) over a new outer dim."""
    return bass.AP(tensor=x.tensor, offset=x.offset,
                   ap=[x.ap[0], [0, r]] + list(x.ap[1:]))


def _bc_in(x, r):
    """[P, n] -> [P, n, r]: broadcast with the step-0 dim innermost."""
    return bass.AP(tensor=x.tensor, offset=x.offset,
                   ap=list(x.ap) + [[0, r]])


def build_nc():
    nc = bacc.Bacc("TRN2", target_bir_lowering=False, debug=False,
                   num_devices=NCORES)
    # box planes (bf16) [P, 19, F]:
    #   0-3  X4 = px0 px1 tx0 tx1      4-7  W4 = pw0 pw1 tw0 tw1
    #   8-11 Y4 = py0 py1 ty0 ty1     12-15 H4 = ph0 ph1 th0 th1
    #   16-18 C3 = pc0 pc1 conf
    box = nc.dram_tensor("box", [P, 19, F], bf16, kind="ExternalInput")
    # class streams (fp8 e3m4), negated target; V part channel-major,
    # G part cell-major.
    cvp = nc.dram_tensor("cvp", [NCQ, P, CHV, MQ], f8e3, kind="ExternalInput")
    cvn = nc.dram_tensor("cvn", [NCQ, P, CHV, MQ], f8e3, kind="ExternalInput")
    if not DISABLE_CLS_G:
        cgp = nc.dram_tensor("cgp", [NCQ, P, MQ, CHG], f8e3,
                             kind="ExternalInput")
        cgn = nc.dram_tensor("cgn", [NCQ, P, MQ, CHG], f8e3,
                             kind="ExternalInput")
    out = nc.dram_tensor("acc_out", [P, NACC], f32, kind="ExternalOutput")

    V = nc.vector
    A = nc.scalar
    G = nc.gpsimd

    with tile.TileContext(nc) as tc:
        with (
            tc.tile_pool(name="inp", bufs=1) as inp,
            tc.tile_pool(name="cls", bufs=2) as clsb,
            tc.tile_pool(name="wk", bufs=1) as wk,
            tc.tile_pool(name="one", bufs=1) as one,
        ):
            acc = one.tile([P, NACC], f32)
            V.memset(acc, 0.0)

            # ---- box DMAs first: V work starts as soon as bxa lands ----
            bxa = inp.tile([P, 8, F], bf16, tag="bxa")   # X4 W4
            nc.sync.dma_start(bxa, box.ap()[:, 0:8])
            bxb = inp.tile([P, 8, F], bf16, tag="bxb")   # Y4 H4
            nc.sync.dma_start(bxb, box.ap()[:, 8:16])
            bxc = inp.tile([P, 3, F], bf16, tag="bxc")   # C3
            nc.sync.dma_start(bxc, box.ap()[:, 16:19])

            # ---- class diff streams: SWDGE cast + CCE add (d = p - t) ----
            cv, cg, cv_dep, cg_dep = [], [], [], []
            for q in range(NCQ if not DISABLE_CLS else 0):
                # CCE accumulate is limited to 2048 elems/partition/DMA:
                # dv is 8*196=1568 (one call); dg is 196*12 - split in two.
                dv = clsb.tile([P, CHV, MQ], bf16, tag="dv")
                G.dma_start(dv, cvp.ap()[q])
                G.dma_start(dv, cvn.ap()[q], accum_op=Alu.add)
                cv.append(dv)
                dg = clsb.tile([P, MQ, CHG], bf16, tag="dg")
                G.dma_start(dg, cgp.ap()[q])
                h = MQ // 2
                G.dma_start(dg[:, 0:h], cgp.ap()[q][:, 0:h],
                            accum_op=Alu.bypass) if False else None
                G.dma_start(dg[:, 0:h], cgn.ap()[q][:, 0:h],
                            accum_op=Alu.add)
                G.dma_start(dg[:, h:MQ], cgn.ap()[q][:, h:MQ],
                            accum_op=Alu.add)
                cg.append(dg)

            X, W = bxa[:, 0:4], bxa[:, 4:8]
            Y, H = bxb[:, 0:4], bxb[:, 4:8]
            pc = bxc[:, 0:2]
            conf = bxc[:, 2]

            # squared-loss ingredients [dx(2) dy(2) dw(2) dh(2) do(2) nb(2)]
            dsq = wk.tile([P, 6, 2, F], bf16, tag="dsq")

            # dx_b = px_b - tx0 (slot-0 target; dsq[0,1] re-done per slot later)
            V.tensor_tensor(dsq[:, 0], X[:, 0:2], _bc(X[:, 2], 2),
                            op=Alu.subtract)
            V.tensor_tensor(dsq[:, 1], Y[:, 0:2], _bc(Y[:, 2], 2),
                            op=Alu.subtract)
            adx = wk.tile([P, 2, 2, F], bf16, tag="adx")
            A.activation(adx[:, 0], dsq[:, 0], Act.Abs)
            A.activation(adx[:, 1], dsq[:, 1], Act.Abs)

            # u = 3.5*(pw_b + tw0) - |dx|
            u = wk.tile([P, 2, 2, F], bf16, tag="u")
            V.tensor_tensor(u[:, 0], W[:, 0:2], _bc(W[:, 2], 2), op=Alu.add)
            V.tensor_tensor(u[:, 1], H[:, 0:2], _bc(H[:, 2], 2), op=Alu.add)
            V.tensor_scalar(u, u, HS, None, op0=Alu.mult)
            V.tensor_tensor(u, u, adx, op=Alu.subtract)

            # m7 = 7 * min(pw_b, tw0); overlap = relu(min(u, m7))
            m7 = wk.tile([P, 2, 2, F], bf16, tag="m7")
            V.tensor_tensor(m7[:, 0], W[:, 0:2], _bc(W[:, 2], 2), op=Alu.min)
            V.tensor_tensor(m7[:, 1], H[:, 0:2], _bc(H[:, 2], 2), op=Alu.min)
            V.tensor_scalar(m7, m7, float(S), None, op0=Alu.mult)
            whr = wk.tile([P, 2, 2, F], bf16, tag="whr")
            V.tensor_tensor(whr, u, m7, op=Alu.min)
            V.tensor_scalar(whr, whr, 0.0, None, op0=Alu.max)

            inter = wk.tile([P, 2, F], bf16, tag="inter")
            V.tensor_tensor(inter, whr[:, 0], whr[:, 1], op=Alu.mult)

            # areas (x S^2): [ap0 ap1 at]
            ar = wk.tile([P, 3, F], bf16, tag="ar")
            V.tensor_scalar(ar, W[:, 0:3], S2, None, op0=Alu.mult)
            V.tensor_tensor(ar, ar, H[:, 0:3], op=Alu.mult)
            den = wk.tile([P, 2, F], bf16, tag="den")
            V.tensor_tensor(den, ar[:, 0:2], _bc(ar[:, 2], 2), op=Alu.add)
            V.tensor_tensor(den, den, inter, op=Alu.subtract)
            den32 = wk.tile([P, 2, F], f32, tag="den32")
            A.activation(den32, den, Act.Copy)
            rden = wk.tile([P, 2, F], f32, tag="rden")
            V.reciprocal_approx_fast(rden, den32)
            rden16 = wk.tile([P, 2, F], bf16, tag="rden16")
            A.activation(rden16, rden, Act.Copy)
            iou = wk.tile([P, 2, F], bf16, tag="iou")
            V.tensor_tensor(iou, inter, rden16, op=Alu.mult)

            # responsibility selection (argmax ties -> box0, like jnp)
            ge = wk.tile([P, F], bf16, tag="ge")
            V.tensor_tensor(ge, iou[:, 0], iou[:, 1], op=Alu.is_ge)
            miou = wk.tile([P, F], bf16, tag="miou")
            V.tensor_tensor(miou, iou[:, 0], iou[:, 1], op=Alu.max)
            resp = wk.tile([P, 2, F], bf16, tag="resp")
            V.tensor_tensor(resp[:, 0], ge, conf, op=Alu.mult)
            V.tensor_tensor(resp[:, 1], conf, resp[:, 0], op=Alu.subtract)

            # wh needs sqrt; nm = 1 - conf
            sq = wk.tile([P, 2, 4, F], bf16, tag="sq")
            A.activation(sq[:, 0], W, Act.Sqrt)
            A.activation(sq[:, 1], H, Act.Sqrt)
            nm = wk.tile([P, F], bf16, tag="nm")
            V.tensor_scalar(nm, conf, -1.0, 1.0, op0=Alu.mult, op1=Alu.add)

            # fix dx/dy box1 to slot-matched target, fill dw dh do nb
            V.tensor_tensor(dsq[:, 0, 1], X[:, 1], X[:, 3], op=Alu.subtract)
            V.tensor_tensor(dsq[:, 1, 1], Y[:, 1], Y[:, 3], op=Alu.subtract)
            V.tensor_tensor(dsq[:, 2], sq[:, 0, 0:2], sq[:, 0, 2:4],
                            op=Alu.subtract)
            V.tensor_tensor(dsq[:, 3], sq[:, 1, 0:2], sq[:, 1, 2:4],
                            op=Alu.subtract)
            V.tensor_tensor(dsq[:, 4], pc, _bc(miou, 2), op=Alu.subtract)
            V.tensor_tensor(dsq[:, 5], pc, _bc(nm, 2), op=Alu.mult)
            # mask xy/wh/obj by responsibility (resp^2 == resp)
            V.tensor_tensor(dsq[:, 0:5], dsq[:, 0:5], _bc(resp, 5),
                            op=Alu.mult)

            # reduction passes (sum of squares per term)
            A.activation(dsq[:, 0:2], dsq[:, 0:2], Act.Square,
                         accum_out=acc[:, 0:1])
            A.activation(dsq[:, 2:4], dsq[:, 2:4], Act.Square,
                         accum_out=acc[:, 1:2])
            A.activation(dsq[:, 4], dsq[:, 4], Act.Square,
                         accum_out=acc[:, 2:3])
            A.activation(dsq[:, 5], dsq[:, 5], Act.Square,
                         accum_out=acc[:, 3:4])

            # class: mask by obj then square+accumulate
            for q in range(NCQ if not DISABLE_CLS else 0):
                cq = conf[:, q * MQ:(q + 1) * MQ]
                V.tensor_tensor(cv[q], cv[q], _bc(cq, CHV), op=Alu.mult)
                A.activation(cv[q], cv[q], Act.Square,
                             accum_out=acc[:, 4 + 2 * q:5 + 2 * q])
                if not DISABLE_CLS_G:
                    G.tensor_tensor(cg[q], cg[q], _bc_in(cq, CHG),
                                     op=Alu.mult)
                    A.activation(cg[q], cg[q], Act.Square,
                                 accum_out=acc[:, 5 + 2 * q:6 + 2 * q])

            nc.sync.dma_start(out.ap(), acc)

    nc.compile()
    return nc


_NC_CACHE = None


def _get_nc():
    global _NC_CACHE
    if _NC_CACHE is None:
        _NC_CACHE = build_nc()
    return _NC_CACHE


# box plane order: (src, channel): X4, W4, Y4, H4, C3
_PLANES = [(0, 0), (0, 5), (1, 0), (1, 5),
           (0, 2), (0, 7), (1, 2), (1, 7),
           (0, 1), (0, 6), (1, 1), (1, 6),
           (0, 3), (0, 8), (1, 3), (1, 8),
           (0, 4), (0, 9), (1, 4)]


def shard_inputs(pred_tensor, target_tensor):
    """Full [16384,7,7,30] f32 -> per-core planar bf16 box + fp8 class."""
    p = np.asarray(pred_tensor, dtype=np.float32).reshape(NCORES, P, F, 30)
    t = np.asarray(target_tensor, dtype=np.float32).reshape(NCORES, P, F, 30)
    src = (p, t)

    box = np.empty((NCORES, P, 19, F), dtype=ml_dtypes.bfloat16)
    for i, (s, ch) in enumerate(_PLANES):
        box[:, :, i] = src[s][..., ch]

    pv = p[..., 10:10 + CHV].astype(ml_dtypes.float8_e3m4)
    nv = (-t[..., 10:10 + CHV]).astype(ml_dtypes.float8_e3m4)
    pg = p[..., 10 + CHV:30].astype(ml_dtypes.float8_e3m4)
    ng = (-t[..., 10 + CHV:30]).astype(ml_dtypes.float8_e3m4)

    def v_pack(x):  # [NCORES, P, F, CHV] -> [NCORES, NCQ, P, CHV, MQ]
        y = x.transpose(0, 1, 3, 2).reshape(NCORES, P, CHV, NCQ, MQ)
        return np.ascontiguousarray(y.transpose(0, 3, 1, 2, 4))

    def g_pack(x):  # [NCORES, P, F, CHG] -> [NCORES, NCQ, P, MQ, CHG]
        y = x.reshape(NCORES, P, NCQ, MQ, CHG)
        return np.ascontiguousarray(y.transpose(0, 2, 1, 3, 4))

    cvp, cvn = v_pack(pv), v_pack(nv)
    if pg.shape[-1]:
        cgp, cgn = g_pack(pg), g_pack(ng)
    else:
        cgp = np.zeros((NCORES, NCQ, P, MQ, CHG), dtype=ml_dtypes.float8_e3m4)
        cgn = cgp
    maps = [{"box": box[c], "cvp": cvp[c], "cvn": cvn[c]}
            for c in range(NCORES)]
    if not DISABLE_CLS_G:
        for c in range(NCORES):
            maps[c]["cgp"] = cgp[c]
            maps[c]["cgn"] = cgn[c]
    return maps


def combine(results):
    """Per-core acc_out [P, NACC] -> 5-tuple of loss scalars."""
    total = np.zeros(5, dtype=np.float64)
    for r in results:
        a = r["acc_out"].astype(np.float64).sum(axis=0)
        total[:4] += a[:4]
        total[4] += a[4:].sum()
    total /= BATCH
    return tuple(np.float32(v) for v in total)


def kernel(pred_tensor, target_tensor):
    nc = _get_nc()
    in_maps = shard_inputs(pred_tensor, target_tensor)
    res = run_bass_kernel_spmd(nc, in_maps, core_ids=list(range(NCORES)))
    return combine(res.results)


# revision 23
# speedup vs baseline: 1.8863x; 1.1225x over previous
"""YOLO-v1-style loss on 8 Trainium2 NeuronCores (Bass/Tile), v3.

Data-parallel over batch: each core gets 2048 of 16384 batch elements
(100,352 cells as 128 partitions x 784 cells); per-partition partial sums
for the 5 loss terms are combined on the host.

Layout: host repacks channels into per-channel planes ([P, plane, cells])
so DVE tensor_tensor ops run dense step-1 bf16 at 2x mode and
tensor_scalar ops at 4x. scalar_tensor_tensor (1x only) is avoided.

IoU via the overlap identity (no corner materialization):
  overlap_x = min(3.5*(pw+tw) - |px-tx|, 7*min(pw,tw)), clamped at 0.

Engine split:
  - DVE: box pipeline + responsibility masks + 8 class channels
  - ACT: sqrt, f32 conversions for reciprocal, square+accumulate passes
  - Pool: 12 class channels (cell-major tile: broadcast mask has its
    step-0 dim innermost - the only fast Q7 pattern) + SWDGE DMA descr gen
  - DMA CCE: class diff (p - t) computed inline: p streams in with an
    fp8->bf16 cast, host-negated t accumulates with cce add.

Explicit add_dep_helper edges order the CCE chain against its readers
(Tile's shadow tracking under-waits on multi-DMA read-modify-write tiles).

Self-contained: hardcodes all shapes; needs numpy + ml_dtypes + concourse.
"""

import numpy as np
import ml_dtypes

import concourse.bass as bass
import concourse.bacc as bacc
import concourse.tile as tile
import concourse.mybir as mybir
from concourse.bass_utils import run_bass_kernel_spmd
from bass_rust import add_dep_helper

DISABLE_CLS = False
DISABLE_CLS_G = False

f32 = mybir.dt.float32
bf16 = mybir.dt.bfloat16
f8e3 = mybir.dt.float8e3
Alu = mybir.AluOpType
Act = mybir.ActivationFunctionType

S = 7
BATCH = 16384
NCORES = 8
PER = BATCH // NCORES          # 2048 batch elems per core
P = 128                        # partitions
F = PER * S * S // P           # 784 cells per partition
NCQ = 4                        # class cell chunks
MQ = F // NCQ                  # 392
CHV = 8                        # class channels masked on DVE (channel-major)
CHG = 20 - CHV                 # class channels masked on Pool (cell-major)
HS = S / 2.0                   # 3.5
S2 = float(S * S)              # 49

NACC = 4 + NCQ


def _bc(x, r):
    """[P, ...] -> [P, r, ...]: broadcast (step-0) over a new outer dim."""
    return bass.AP(tensor=x.tensor, offset=x.offset,
                   ap=[x.ap[0], [0, r]] + list(x.ap[1:]))


def _bc_in(x, r):
    """[P, n] -> [P, n, r]: broadcast with the step-0 dim innermost."""
    return bass.AP(tensor=x.tensor, offset=x.offset,
                   ap=list(x.ap) + [[0, r]])


def build_nc():
    nc = bacc.Bacc("TRN2", target_bir_lowering=False, debug=False,
                   num_devices=NCORES)
    # box planes (bf16) [P, 19, F]:
    #   0-3  X4 = px0 px1 tx0 tx1      4-7  W4 = pw0 pw1 tw0 tw1
    #   8-11 Y4 = py0 py1 ty0 ty1     12-15 H4 = ph0 ph1 th0 th1
    #   16-18 C3 = pc0 pc1 conf
    box = nc.dram_tensor("box", [P, 19, F], bf16, kind="ExternalInput")
    # class streams (fp8 e3m4), negated target; V part channel-major,
    # G part cell-major.
    cvp = nc.dram_tensor("cvp", [NCQ, P, 20, MQ], f8e3, kind="ExternalInput")
    cvn = nc.dram_tensor("cvn", [NCQ, P, 20, MQ], f8e3, kind="ExternalInput")

    out = nc.dram_tensor("acc_out", [P, NACC], f32, kind="ExternalOutput")

    V = nc.vector
    A = nc.scalar
    G = nc.gpsimd

    with tile.TileContext(nc) as tc:
        with (
            tc.tile_pool(name="inp", bufs=1) as inp,
            tc.tile_pool(name="cls", bufs=2) as clsb,
            tc.tile_pool(name="wk", bufs=1) as wk,
            tc.tile_pool(name="one", bufs=1) as one,
        ):
            acc = one.tile([P, NACC], f32)
            V.memset(acc, 0.0)

            # ---- box DMAs first: V work starts as soon as X4 lands ----
            bxa = inp.tile([P, 8, F], bf16, tag="bxa")   # X4 W4
            nc.sync.dma_start(bxa[:, 0:4], box.ap()[:, 0:4])
            nc.sync.dma_start(bxa[:, 4:8], box.ap()[:, 4:8])
            bxb = inp.tile([P, 8, F], bf16, tag="bxb")   # Y4 H4
            nc.sync.dma_start(bxb[:, 0:4], box.ap()[:, 8:12])
            nc.sync.dma_start(bxb[:, 4:8], box.ap()[:, 12:16])
            bxc = inp.tile([P, 3, F], bf16, tag="bxc")   # C3
            nc.sync.dma_start(bxc, box.ap()[:, 16:19])

            # ---- class diff stream: SWDGE cast + CCE add (d = p - t) ----
            # CCE accumulate caps at 2048 elems/partition per DMA: the
            # cast moves all 20 channels, the accumulate goes in 2 halves.
            cv = []
            for q in range(NCQ):
                dv = clsb.tile([P, 20, MQ], bf16, tag="dv")
                G.dma_start(dv, cvp.ap()[q])
                G.dma_start(dv[:, 0:10], cvn.ap()[q][:, 0:10],
                            accum_op=Alu.add)
                G.dma_start(dv[:, 10:20], cvn.ap()[q][:, 10:20],
                            accum_op=Alu.add)
                cv.append(dv)

            X, W = bxa[:, 0:4], bxa[:, 4:8]
            Y, H = bxb[:, 0:4], bxb[:, 4:8]
            pc = bxc[:, 0:2]
            conf = bxc[:, 2]

            # squared-loss ingredients [dx(2) dy(2) dw(2) dh(2) do(2) nb(2)]
            dsq = wk.tile([P, 6, 2, F], bf16, tag="dsq")

            # dx_b = px_b - tx0 (slot-0 target; dsq[0,1] re-done per slot later)
            V.tensor_tensor(dsq[:, 0], X[:, 0:2], _bc(X[:, 2], 2),
                            op=Alu.subtract)
            V.tensor_tensor(dsq[:, 1], Y[:, 0:2], _bc(Y[:, 2], 2),
                            op=Alu.subtract)
            adx = wk.tile([P, 2, 2, F], bf16, tag="adx")
            A.activation(adx[:, 0], dsq[:, 0], Act.Abs)
            A.activation(adx[:, 1], dsq[:, 1], Act.Abs)

            # u = 3.5*(pw_b + tw0) - |dx|
            u = wk.tile([P, 2, 2, F], bf16, tag="u")
            V.tensor_tensor(u[:, 0], W[:, 0:2], _bc(W[:, 2], 2), op=Alu.add)
            V.tensor_tensor(u[:, 1], H[:, 0:2], _bc(H[:, 2], 2), op=Alu.add)
            V.tensor_scalar(u, u, HS, None, op0=Alu.mult)
            V.tensor_tensor(u, u, adx, op=Alu.subtract)

            # m7 = 7 * min(pw_b, tw0); overlap = relu(min(u, m7))
            m7 = wk.tile([P, 2, 2, F], bf16, tag="m7")
            V.tensor_tensor(m7[:, 0], W[:, 0:2], _bc(W[:, 2], 2), op=Alu.min)
            V.tensor_tensor(m7[:, 1], H[:, 0:2], _bc(H[:, 2], 2), op=Alu.min)
            V.tensor_scalar(m7, m7, float(S), None, op0=Alu.mult)
            whr = wk.tile([P, 2, 2, F], bf16, tag="whr")
            V.tensor_tensor(whr, u, m7, op=Alu.min)
            V.tensor_scalar(whr, whr, 0.0, None, op0=Alu.max)

            inter = wk.tile([P, 2, F], bf16, tag="inter")
            V.tensor_tensor(inter, whr[:, 0], whr[:, 1], op=Alu.mult)

            # areas (x S^2): [ap0 ap1 at]
            ar = wk.tile([P, 3, F], bf16, tag="ar")
            V.tensor_scalar(ar, W[:, 0:3], S2, None, op0=Alu.mult)
            V.tensor_tensor(ar, ar, H[:, 0:3], op=Alu.mult)
            den = wk.tile([P, 2, F], bf16, tag="den")
            V.tensor_tensor(den, ar[:, 0:2], _bc(ar[:, 2], 2), op=Alu.add)
            V.tensor_tensor(den, den, inter, op=Alu.subtract)
            den32 = wk.tile([P, 2, F], f32, tag="den32")
            V.tensor_copy(den32, den)
            rden = wk.tile([P, 2, F], f32, tag="rden")
            V.reciprocal_approx_fast(rden, den32)
            rden16 = wk.tile([P, 2, F], bf16, tag="rden16")
            V.tensor_copy(rden16, rden)
            iou = wk.tile([P, 2, F], bf16, tag="iou")
            V.tensor_tensor(iou, inter, rden16, op=Alu.mult)

            # responsibility selection (argmax ties -> box0, like jnp)
            ge = wk.tile([P, F], bf16, tag="ge")
            V.tensor_tensor(ge, iou[:, 0], iou[:, 1], op=Alu.is_ge)
            miou = wk.tile([P, F], bf16, tag="miou")
            V.tensor_tensor(miou, iou[:, 0], iou[:, 1], op=Alu.max)
            resp = wk.tile([P, 2, F], bf16, tag="resp")
            V.tensor_tensor(resp[:, 0], ge, conf, op=Alu.mult)
            V.tensor_tensor(resp[:, 1], conf, resp[:, 0], op=Alu.subtract)

            # wh needs sqrt; nm = 1 - conf
            sq = wk.tile([P, 2, 4, F], bf16, tag="sq")
            A.activation(sq[:, 0], W, Act.Sqrt)
            A.activation(sq[:, 1], H, Act.Sqrt)
            nm = wk.tile([P, F], bf16, tag="nm")
            V.tensor_scalar(nm, conf, -1.0, 1.0, op0=Alu.mult, op1=Alu.add)

            # fix dx/dy box1 to slot-matched target, fill dw dh do nb
            V.tensor_tensor(dsq[:, 0, 1], X[:, 1], X[:, 3], op=Alu.subtract)
            V.tensor_tensor(dsq[:, 1, 1], Y[:, 1], Y[:, 3], op=Alu.subtract)
            V.tensor_tensor(dsq[:, 2], sq[:, 0, 0:2], sq[:, 0, 2:4],
                            op=Alu.subtract)
            V.tensor_tensor(dsq[:, 3], sq[:, 1, 0:2], sq[:, 1, 2:4],
                            op=Alu.subtract)
            V.tensor_tensor(dsq[:, 4], pc, _bc(miou, 2), op=Alu.subtract)
            V.tensor_tensor(dsq[:, 5], pc, _bc(nm, 2), op=Alu.mult)
            # mask xy/wh/obj by responsibility (resp^2 == resp)
            V.tensor_tensor(dsq[:, 0:5], dsq[:, 0:5], _bc(resp, 5),
                            op=Alu.mult)

            # reduction passes (sum of squares per term)
            A.activation(dsq[:, 0:2], dsq[:, 0:2], Act.Square,
                         accum_out=acc[:, 0:1])
            A.activation(dsq[:, 2:4], dsq[:, 2:4], Act.Square,
                         accum_out=acc[:, 1:2])
            A.activation(dsq[:, 4], dsq[:, 4], Act.Square,
                         accum_out=acc[:, 2:3])
            A.activation(dsq[:, 5], dsq[:, 5], Act.Square,
                         accum_out=acc[:, 3:4])

            # class: mask by obj then square+accumulate
            for q in range(NCQ):
                cq = conf[:, q * MQ:(q + 1) * MQ]
                V.tensor_tensor(cv[q], cv[q], _bc(cq, 20), op=Alu.mult)
                A.activation(cv[q], cv[q], Act.Square,
                             accum_out=acc[:, 4 + q:5 + q])

            nc.sync.dma_start(out.ap(), acc)

    nc.compile()
    return nc


_NC_CACHE = None


def _get_nc():
    global _NC_CACHE
    if _NC_CACHE is None:
        _NC_CACHE = build_nc()
    return _NC_CACHE


# box plane order: (src, channel): X4, W4, Y4, H4, C3
_PLANES = [(0, 0), (0, 5), (1, 0), (1, 5),
           (0, 2), (0, 7), (1, 2), (1, 7),
           (0, 1), (0, 6), (1, 1), (1, 6),
           (0, 3), (0, 8), (1, 3), (1, 8),
           (0, 4), (0, 9), (1, 4)]


def shard_inputs(pred_tensor, target_tensor):
    """Full [16384,7,7,30] f32 -> per-core planar bf16 box + fp8 class."""
    p = np.asarray(pred_tensor, dtype=np.float32).reshape(NCORES, P, F, 30)
    t = np.asarray(target_tensor, dtype=np.float32).reshape(NCORES, P, F, 30)
    src = (p, t)

    box = np.empty((NCORES, P, 19, F), dtype=ml_dtypes.bfloat16)
    for i, (s, ch) in enumerate(_PLANES):
        box[:, :, i] = src[s][..., ch]

    pv = p[..., 10:30].astype(ml_dtypes.float8_e3m4)
    nv = (-t[..., 10:30]).astype(ml_dtypes.float8_e3m4)

    def v_pack(x):  # [NCORES, P, F, 20] -> [NCORES, NCQ, P, 20, MQ]
        y = x.transpose(0, 1, 3, 2).reshape(NCORES, P, 20, NCQ, MQ)
        return np.ascontiguousarray(y.transpose(0, 3, 1, 2, 4))

    cvp, cvn = v_pack(pv), v_pack(nv)
    return [{"box": box[c], "cvp": cvp[c], "cvn": cvn[c]}
            for c in range(NCORES)]


def combine(results):
    """Per-core acc_out [P, NACC] -> 5-tuple of loss scalars."""
    total = np.zeros(5, dtype=np.float64)
    for r in results:
        a = r["acc_out"].astype(np.float64).sum(axis=0)
        total[:4] += a[:4]
        total[4] += a[4:].sum()
    total /= BATCH
    return tuple(np.float32(v) for v in total)


def kernel(pred_tensor, target_tensor):
    nc = _get_nc()
    in_maps = shard_inputs(pred_tensor, target_tensor)
    res = run_bass_kernel_spmd(nc, in_maps, core_ids=list(range(NCORES)))
    return combine(res.results)


# revision 24
# speedup vs baseline: 1.9879x; 1.0539x over previous
"""YOLO-v1-style loss on 8 Trainium2 NeuronCores (Bass/Tile), v3.

Data-parallel over batch: each core gets 2048 of 16384 batch elements
(100,352 cells as 128 partitions x 784 cells); per-partition partial sums
for the 5 loss terms are combined on the host.

Layout: host repacks channels into per-channel planes ([P, plane, cells])
so DVE tensor_tensor ops run dense step-1 bf16 at 2x mode and
tensor_scalar ops at 4x. scalar_tensor_tensor (1x only) is avoided.

IoU via the overlap identity (no corner materialization):
  overlap_x = min(3.5*(pw+tw) - |px-tx|, 7*min(pw,tw)), clamped at 0.

Engine split:
  - DVE: box pipeline + responsibility masks + 8 class channels
  - ACT: sqrt, f32 conversions for reciprocal, square+accumulate passes
  - Pool: 12 class channels (cell-major tile: broadcast mask has its
    step-0 dim innermost - the only fast Q7 pattern) + SWDGE DMA descr gen
  - DMA CCE: class diff (p - t) computed inline: p streams in with an
    fp8->bf16 cast, host-negated t accumulates with cce add.

Explicit add_dep_helper edges order the CCE chain against its readers
(Tile's shadow tracking under-waits on multi-DMA read-modify-write tiles).

Self-contained: hardcodes all shapes; needs numpy + ml_dtypes + concourse.
"""

import numpy as np
import ml_dtypes

import concourse.bass as bass
import concourse.bacc as bacc
import concourse.tile as tile
import concourse.mybir as mybir
from concourse.bass_utils import run_bass_kernel_spmd
from bass_rust import add_dep_helper

DISABLE_CLS = False
DISABLE_CLS_G = False

f32 = mybir.dt.float32
bf16 = mybir.dt.bfloat16
f8e3 = mybir.dt.float8e3
Alu = mybir.AluOpType
Act = mybir.ActivationFunctionType

S = 7
BATCH = 16384
NCORES = 8
PER = BATCH // NCORES          # 2048 batch elems per core
P = 128                        # partitions
F = PER * S * S // P           # 784 cells per partition
NCQ = 4                        # class cell chunks
MQ = F // NCQ                  # 392
CHV = 8                        # class channels masked on DVE (channel-major)
CHG = 20 - CHV                 # class channels masked on Pool (cell-major)
HS = S / 2.0                   # 3.5
S2 = float(S * S)              # 49

NACC = 4 + NCQ


def _bc(x, r):
    """[P, ...] -> [P, r, ...]: broadcast (step-0) over a new outer dim."""
    return bass.AP(tensor=x.tensor, offset=x.offset,
                   ap=[x.ap[0], [0, r]] + list(x.ap[1:]))


def _bc_in(x, r):
    """[P, n] -> [P, n, r]: broadcast with the step-0 dim innermost."""
    return bass.AP(tensor=x.tensor, offset=x.offset,
                   ap=list(x.ap) + [[0, r]])


def build_nc():
    nc = bacc.Bacc("TRN2", target_bir_lowering=False, debug=False,
                   num_devices=NCORES)
    # box planes (bf16) [P, 19, F]:
    #   0-3  X4 = px0 px1 tx0 tx1      4-7  W4 = pw0 pw1 tw0 tw1
    #   8-11 Y4 = py0 py1 ty0 ty1     12-15 H4 = ph0 ph1 th0 th1
    #   16-18 C3 = pc0 pc1 conf
    box = nc.dram_tensor("box", [P, 19, F], bf16, kind="ExternalInput")
    # class streams (fp8 e3m4), negated target; V part channel-major,
    # G part cell-major.
    cvp = nc.dram_tensor("cvp", [NCQ, P, 20, MQ], f8e3, kind="ExternalInput")
    cvn = nc.dram_tensor("cvn", [NCQ, P, 20, MQ], f8e3, kind="ExternalInput")

    out = nc.dram_tensor("acc_out", [P, NACC], f32, kind="ExternalOutput")

    V = nc.vector
    A = nc.scalar
    G = nc.gpsimd

    with tile.TileContext(nc) as tc:
        with (
            tc.tile_pool(name="inp", bufs=1) as inp,
            tc.tile_pool(name="cls", bufs=4) as clsb,
            tc.tile_pool(name="wk", bufs=1) as wk,
            tc.tile_pool(name="one", bufs=1) as one,
        ):
            acc = one.tile([P, NACC], f32)
            V.memset(acc, 0.0)

            # ---- box DMAs first: V work starts as soon as X4 lands ----
            bxa = inp.tile([P, 8, F], bf16, tag="bxa")   # X4 W4
            nc.sync.dma_start(bxa[:, 0:4], box.ap()[:, 0:4])
            i_w4 = nc.sync.dma_start(bxa[:, 4:8], box.ap()[:, 4:8])
            bxb = inp.tile([P, 8, F], bf16, tag="bxb")   # Y4 H4
            nc.sync.dma_start(bxb[:, 0:4], box.ap()[:, 8:12])
            i_h4 = nc.sync.dma_start(bxb[:, 4:8], box.ap()[:, 12:16])
            bxc = inp.tile([P, 3, F], bf16, tag="bxc")   # C3
            i_c3 = nc.sync.dma_start(bxc, box.ap()[:, 16:19])
            box_gate = [i_w4, i_h4, i_c3, i_c3]

            # ---- class diff stream: SWDGE cast + CCE add (d = p - t) ----
            # CCE accumulate caps at 2048 elems/partition per DMA: the
            # cast moves all 20 channels, the accumulate goes in 2 halves.
            cv = []
            for q in range(NCQ):
                dv = clsb.tile([P, 20, MQ], bf16, tag="dv")
                ic = G.dma_start(dv, cvp.ap()[q])
                # throttle class stream behind the box DMAs so the box
                # pipeline's inputs land at full HBM rate first
                add_dep_helper(ic.ins, box_gate[q].ins,
                               reason="cls stream after box stream")
                G.dma_start(dv[:, 0:10], cvn.ap()[q][:, 0:10],
                            accum_op=Alu.add)
                G.dma_start(dv[:, 10:20], cvn.ap()[q][:, 10:20],
                            accum_op=Alu.add)
                cv.append(dv)

            X, W = bxa[:, 0:4], bxa[:, 4:8]
            Y, H = bxb[:, 0:4], bxb[:, 4:8]
            pc = bxc[:, 0:2]
            conf = bxc[:, 2]

            # squared-loss ingredients [dx(2) dy(2) dw(2) dh(2) do(2) nb(2)]
            dsq = wk.tile([P, 6, 2, F], bf16, tag="dsq")

            # dx_b = px_b - tx0 (slot-0 target; dsq[0,1] re-done per slot later)
            V.tensor_tensor(dsq[:, 0], X[:, 0:2], _bc(X[:, 2], 2),
                            op=Alu.subtract)
            V.tensor_tensor(dsq[:, 1], Y[:, 0:2], _bc(Y[:, 2], 2),
                            op=Alu.subtract)
            adx = wk.tile([P, 2, 2, F], bf16, tag="adx")
            A.activation(adx[:, 0], dsq[:, 0], Act.Abs)
            A.activation(adx[:, 1], dsq[:, 1], Act.Abs)

            # u = 3.5*(pw_b + tw0) - |dx|
            u = wk.tile([P, 2, 2, F], bf16, tag="u")
            V.tensor_tensor(u[:, 0], W[:, 0:2], _bc(W[:, 2], 2), op=Alu.add)
            V.tensor_tensor(u[:, 1], H[:, 0:2], _bc(H[:, 2], 2), op=Alu.add)
            V.tensor_scalar(u, u, HS, None, op0=Alu.mult)
            V.tensor_tensor(u, u, adx, op=Alu.subtract)

            # m7 = 7 * min(pw_b, tw0); overlap = relu(min(u, m7))
            m7 = wk.tile([P, 2, 2, F], bf16, tag="m7")
            V.tensor_tensor(m7[:, 0], W[:, 0:2], _bc(W[:, 2], 2), op=Alu.min)
            V.tensor_tensor(m7[:, 1], H[:, 0:2], _bc(H[:, 2], 2), op=Alu.min)
            V.tensor_scalar(m7, m7, float(S), None, op0=Alu.mult)
            whr = wk.tile([P, 2, 2, F], bf16, tag="whr")
            V.tensor_tensor(whr, u, m7, op=Alu.min)
            V.tensor_scalar(whr, whr, 0.0, None, op0=Alu.max)

            inter = wk.tile([P, 2, F], bf16, tag="inter")
            V.tensor_tensor(inter, whr[:, 0], whr[:, 1], op=Alu.mult)

            # areas (x S^2): [ap0 ap1 at]
            ar = wk.tile([P, 3, F], bf16, tag="ar")
            V.tensor_scalar(ar, W[:, 0:3], S2, None, op0=Alu.mult)
            V.tensor_tensor(ar, ar, H[:, 0:3], op=Alu.mult)
            den = wk.tile([P, 2, F], bf16, tag="den")
            V.tensor_tensor(den, ar[:, 0:2], _bc(ar[:, 2], 2), op=Alu.add)
            V.tensor_tensor(den, den, inter, op=Alu.subtract)
            den32 = wk.tile([P, 2, F], f32, tag="den32")
            V.tensor_copy(den32, den)
            rden = wk.tile([P, 2, F], f32, tag="rden")
            V.reciprocal_approx_fast(rden, den32)
            rden16 = wk.tile([P, 2, F], bf16, tag="rden16")
            V.tensor_copy(rden16, rden)
            iou = wk.tile([P, 2, F], bf16, tag="iou")
            V.tensor_tensor(iou, inter, rden16, op=Alu.mult)

            # responsibility selection (argmax ties -> box0, like jnp)
            ge = wk.tile([P, F], bf16, tag="ge")
            V.tensor_tensor(ge, iou[:, 0], iou[:, 1], op=Alu.is_ge)
            miou = wk.tile([P, F], bf16, tag="miou")
            V.tensor_tensor(miou, iou[:, 0], iou[:, 1], op=Alu.max)
            resp = wk.tile([P, 2, F], bf16, tag="resp")
            V.tensor_tensor(resp[:, 0], ge, conf, op=Alu.mult)
            V.tensor_tensor(resp[:, 1], conf, resp[:, 0], op=Alu.subtract)

            # wh needs sqrt; nm = 1 - conf
            sq = wk.tile([P, 2, 4, F], bf16, tag="sq")
            A.activation(sq[:, 0], W, Act.Sqrt)
            A.activation(sq[:, 1], H, Act.Sqrt)
            nm = wk.tile([P, F], bf16, tag="nm")
            V.tensor_scalar(nm, conf, -1.0, 1.0, op0=Alu.mult, op1=Alu.add)

            # fix dx/dy box1 to slot-matched target, fill dw dh do nb
            V.tensor_tensor(dsq[:, 0, 1], X[:, 1], X[:, 3], op=Alu.subtract)
            V.tensor_tensor(dsq[:, 1, 1], Y[:, 1], Y[:, 3], op=Alu.subtract)
            V.tensor_tensor(dsq[:, 2], sq[:, 0, 0:2], sq[:, 0, 2:4],
                            op=Alu.subtract)
            V.tensor_tensor(dsq[:, 3], sq[:, 1, 0:2], sq[:, 1, 2:4],
                            op=Alu.subtract)
            V.tensor_tensor(dsq[:, 4], pc, _bc(miou, 2), op=Alu.subtract)
            V.tensor_tensor(dsq[:, 5], pc, _bc(nm, 2), op=Alu.mult)
            # mask xy/wh/obj by responsibility (resp^2 == resp)
            V.tensor_tensor(dsq[:, 0:5], dsq[:, 0:5], _bc(resp, 5),
                            op=Alu.mult)

            # reduction passes (sum of squares per term)
            A.activation(dsq[:, 0:2], dsq[:, 0:2], Act.Square,
                         accum_out=acc[:, 0:1])
            A.activation(dsq[:, 2:4], dsq[:, 2:4], Act.Square,
                         accum_out=acc[:, 1:2])
            A.activation(dsq[:, 4], dsq[:, 4], Act.Square,
                         accum_out=acc[:, 2:3])
            A.activation(dsq[:, 5], dsq[:, 5], Act.Square,
                         accum_out=acc[:, 3:4])

            # class: mask by obj then square+accumulate
            for q in range(NCQ):
                cq = conf[:, q * MQ:(q + 1) * MQ]
                V.tensor_tensor(cv[q], cv[q], _bc(cq, 20), op=Alu.mult)
                A.activation(cv[q], cv[q], Act.Square,
                             accum_out=acc[:, 4 + q:5 + q])

            nc.sync.dma_start(out.ap(), acc)

    nc.compile()
    return nc


_NC_CACHE = None


def _get_nc():
    global _NC_CACHE
    if _NC_CACHE is None:
        _NC_CACHE = build_nc()
    return _NC_CACHE


# box plane order: (src, channel): X4, W4, Y4, H4, C3
_PLANES = [(0, 0), (0, 5), (1, 0), (1, 5),
           (0, 2), (0, 7), (1, 2), (1, 7),
           (0, 1), (0, 6), (1, 1), (1, 6),
           (0, 3), (0, 8), (1, 3), (1, 8),
           (0, 4), (0, 9), (1, 4)]


def shard_inputs(pred_tensor, target_tensor):
    """Full [16384,7,7,30] f32 -> per-core planar bf16 box + fp8 class."""
    p = np.asarray(pred_tensor, dtype=np.float32).reshape(NCORES, P, F, 30)
    t = np.asarray(target_tensor, dtype=np.float32).reshape(NCORES, P, F, 30)
    src = (p, t)

    box = np.empty((NCORES, P, 19, F), dtype=ml_dtypes.bfloat16)
    for i, (s, ch) in enumerate(_PLANES):
        box[:, :, i] = src[s][..., ch]

    pv = p[..., 10:30].astype(ml_dtypes.float8_e3m4)
    nv = (-t[..., 10:30]).astype(ml_dtypes.float8_e3m4)

    def v_pack(x):  # [NCORES, P, F, 20] -> [NCORES, NCQ, P, 20, MQ]
        y = x.transpose(0, 1, 3, 2).reshape(NCORES, P, 20, NCQ, MQ)
        return np.ascontiguousarray(y.transpose(0, 3, 1, 2, 4))

    cvp, cvn = v_pack(pv), v_pack(nv)
    return [{"box": box[c], "cvp": cvp[c], "cvn": cvn[c]}
            for c in range(NCORES)]


def combine(results):
    """Per-core acc_out [P, NACC] -> 5-tuple of loss scalars."""
    total = np.zeros(5, dtype=np.float64)
    for r in results:
        a = r["acc_out"].astype(np.float64).sum(axis=0)
        total[:4] += a[:4]
        total[4] += a[4:].sum()
    total /= BATCH
    return tuple(np.float32(v) for v in total)


def kernel(pred_tensor, target_tensor):
    nc = _get_nc()
    in_maps = shard_inputs(pred_tensor, target_tensor)
    res = run_bass_kernel_spmd(nc, in_maps, core_ids=list(range(NCORES)))
    return combine(res.results)


# revision 25
# speedup vs baseline: 2.1131x; 1.0630x over previous
"""YOLO-v1-style loss on 8 Trainium2 NeuronCores (Bass/Tile), v3.

Data-parallel over batch: each core gets 2048 of 16384 batch elements
(100,352 cells as 128 partitions x 784 cells); per-partition partial sums
for the 5 loss terms are combined on the host.

Layout: host repacks channels into per-channel planes ([P, plane, cells])
so DVE tensor_tensor ops run dense step-1 bf16 at 2x mode and
tensor_scalar ops at 4x. scalar_tensor_tensor (1x only) is avoided.

IoU via the overlap identity (no corner materialization):
  overlap_x = min(3.5*(pw+tw) - |px-tx|, 7*min(pw,tw)), clamped at 0.

Engine split:
  - DVE: box pipeline + responsibility masks + 8 class channels
  - ACT: sqrt, f32 conversions for reciprocal, square+accumulate passes
  - Pool: 12 class channels (cell-major tile: broadcast mask has its
    step-0 dim innermost - the only fast Q7 pattern) + SWDGE DMA descr gen
  - DMA CCE: class diff (p - t) computed inline: p streams in with an
    fp8->bf16 cast, host-negated t accumulates with cce add.

Explicit add_dep_helper edges order the CCE chain against its readers
(Tile's shadow tracking under-waits on multi-DMA read-modify-write tiles).

Self-contained: hardcodes all shapes; needs numpy + ml_dtypes + concourse.
"""

import numpy as np
import ml_dtypes

import concourse.bass as bass
import concourse.bacc as bacc
import concourse.tile as tile
import concourse.mybir as mybir
from concourse.bass_utils import run_bass_kernel_spmd
from bass_rust import add_dep_helper

DISABLE_CLS = False
DISABLE_CLS_G = False

f32 = mybir.dt.float32
bf16 = mybir.dt.bfloat16
f8e3 = mybir.dt.float8e3
Alu = mybir.AluOpType
Act = mybir.ActivationFunctionType

S = 7
BATCH = 16384
NCORES = 8
PER = BATCH // NCORES          # 2048 batch elems per core
P = 128                        # partitions
F = PER * S * S // P           # 784 cells per partition
NCQ = 4                        # class cell chunks
MQ = F // NCQ                  # 392
CHV = 8                        # class channels masked on DVE (channel-major)
CHG = 20 - CHV                 # class channels masked on Pool (cell-major)
HS = S / 2.0                   # 3.5
S2 = float(S * S)              # 49

NACC = 4 + NCQ


def _bc(x, r):
    """[P, ...] -> [P, r, ...]: broadcast (step-0) over a new outer dim."""
    return bass.AP(tensor=x.tensor, offset=x.offset,
                   ap=[x.ap[0], [0, r]] + list(x.ap[1:]))


def _bc_in(x, r):
    """[P, n] -> [P, n, r]: broadcast with the step-0 dim innermost."""
    return bass.AP(tensor=x.tensor, offset=x.offset,
                   ap=list(x.ap) + [[0, r]])


def build_nc():
    nc = bacc.Bacc("TRN2", target_bir_lowering=False, debug=False,
                   num_devices=NCORES)
    # box planes (bf16) [P, 19, F]:
    #   0-3  X4 = px0 px1 tx0 tx1      4-7  W4 = pw0 pw1 tw0 tw1
    #   8-11 Y4 = py0 py1 ty0 ty1     12-15 H4 = ph0 ph1 th0 th1
    #   16-18 C3 = pc0 pc1 conf
    box = nc.dram_tensor("box", [P, 19, F], bf16, kind="ExternalInput")
    # class streams (fp8 e3m4), negated target; V part channel-major,
    # G part cell-major.
    cvp = nc.dram_tensor("cvp", [NCQ, P, 20, MQ], f8e3, kind="ExternalInput")
    cvn = nc.dram_tensor("cvn", [NCQ, P, 20, MQ], f8e3, kind="ExternalInput")

    out = nc.dram_tensor("acc_out", [P, NACC], f32, kind="ExternalOutput")

    V = nc.vector
    A = nc.scalar
    G = nc.gpsimd

    with tile.TileContext(nc) as tc:
        with (
            tc.tile_pool(name="inp", bufs=1) as inp,
            tc.tile_pool(name="cls", bufs=4) as clsb,
            tc.tile_pool(name="wk", bufs=1) as wk,
            tc.tile_pool(name="one", bufs=1) as one,
        ):
            acc = one.tile([P, NACC], f32)
            V.memset(acc, 0.0)

            # ---- box DMAs first: V work starts as soon as X4 lands ----
            bxa = inp.tile([P, 8, F], bf16, tag="bxa")   # X4 W4
            nc.sync.dma_start(bxa[:, 0:4], box.ap()[:, 0:4])
            i_w4 = nc.sync.dma_start(bxa[:, 4:8], box.ap()[:, 4:8])
            bxb = inp.tile([P, 8, F], bf16, tag="bxb")   # Y4 H4
            nc.sync.dma_start(bxb[:, 0:4], box.ap()[:, 8:12])
            i_h4 = nc.sync.dma_start(bxb[:, 4:8], box.ap()[:, 12:16])
            bxc = inp.tile([P, 3, F], bf16, tag="bxc")   # C3
            i_c3 = nc.sync.dma_start(bxc, box.ap()[:, 16:19])
            box_gate = [i_w4, i_h4, i_c3, i_c3]

            # ---- class diff stream: SWDGE cast + CCE add (d = p - t) ----
            # CCE accumulate caps at 2048 elems/partition per DMA: the
            # cast moves all 20 channels, the accumulate goes in 2 halves.
            cv = []
            for q in range(NCQ):
                dv = clsb.tile([P, 20, MQ], bf16, tag="dv")
                ic = G.dma_start(dv, cvp.ap()[q])
                # throttle class stream behind the box DMAs so the box
                # pipeline's inputs land at full HBM rate first
                add_dep_helper(ic.ins, box_gate[q].ins,
                               reason="cls stream after box stream")
                G.dma_start(dv[:, 0:10], cvn.ap()[q][:, 0:10],
                            accum_op=Alu.add)
                G.dma_start(dv[:, 10:20], cvn.ap()[q][:, 10:20],
                            accum_op=Alu.add)
                cv.append(dv)

            X, W = bxa[:, 0:4], bxa[:, 4:8]
            Y, H = bxb[:, 0:4], bxb[:, 4:8]
            pc = bxc[:, 0:2]
            conf = bxc[:, 2]

            # squared-loss ingredients [dx(2) dy(2) dw(2) dh(2) do(2) nb(2)]
            dsq = wk.tile([P, 6, 2, F], bf16, tag="dsq")

            # dx_b = px_b - tx0 (slot-0 target; dsq[0,1] re-done per slot later)
            V.tensor_tensor(dsq[:, 0], X[:, 0:2], _bc(X[:, 2], 2),
                            op=Alu.subtract)
            V.tensor_tensor(dsq[:, 1], Y[:, 0:2], _bc(Y[:, 2], 2),
                            op=Alu.subtract)
            adx = wk.tile([P, 2, 2, F], bf16, tag="adx")
            A.activation(adx[:, 0], dsq[:, 0], Act.Abs)
            A.activation(adx[:, 1], dsq[:, 1], Act.Abs)

            # u = 3.5*(pw_b + tw0) - |dx|
            u = wk.tile([P, 2, 2, F], bf16, tag="u")
            V.tensor_tensor(u[:, 0], W[:, 0:2], _bc(W[:, 2], 2), op=Alu.add)
            V.tensor_tensor(u[:, 1], H[:, 0:2], _bc(H[:, 2], 2), op=Alu.add)
            V.tensor_scalar(u, u, HS, None, op0=Alu.mult)
            V.tensor_tensor(u, u, adx, op=Alu.subtract)

            # m7 = 7 * min(pw_b, tw0); overlap = relu(min(u, m7))
            m7 = wk.tile([P, 2, 2, F], bf16, tag="m7")
            V.tensor_tensor(m7[:, 0], W[:, 0:2], _bc(W[:, 2], 2), op=Alu.min)
            V.tensor_tensor(m7[:, 1], H[:, 0:2], _bc(H[:, 2], 2), op=Alu.min)
            V.tensor_scalar(m7, m7, float(S), None, op0=Alu.mult)
            whr = wk.tile([P, 2, 2, F], bf16, tag="whr")
            V.tensor_tensor(whr, u, m7, op=Alu.min)
            V.tensor_scalar(whr, whr, 0.0, None, op0=Alu.max)

            inter = wk.tile([P, 2, F], bf16, tag="inter")
            V.tensor_tensor(inter, whr[:, 0], whr[:, 1], op=Alu.mult)

            # areas (x S^2): [ap0 ap1 at]
            ar = wk.tile([P, 3, F], bf16, tag="ar")
            V.tensor_scalar(ar, W[:, 0:3], S2, None, op0=Alu.mult)
            V.tensor_tensor(ar, ar, H[:, 0:3], op=Alu.mult)
            den = wk.tile([P, 2, F], bf16, tag="den")
            V.tensor_tensor(den, ar[:, 0:2], _bc(ar[:, 2], 2), op=Alu.add)
            V.tensor_tensor(den, den, inter, op=Alu.subtract)
            den32 = wk.tile([P, 2, F], f32, tag="den32")
            V.tensor_copy(den32, den)
            rden = wk.tile([P, 2, F], f32, tag="rden")
            V.reciprocal_approx_fast(rden, den32)
            rden16 = wk.tile([P, 2, F], bf16, tag="rden16")
            V.tensor_copy(rden16, rden)
            iou = wk.tile([P, 2, F], bf16, tag="iou")
            V.tensor_tensor(iou, inter, rden16, op=Alu.mult)

            # responsibility selection (argmax ties -> box0, like jnp)
            ge = wk.tile([P, F], bf16, tag="ge")
            V.tensor_tensor(ge, iou[:, 0], iou[:, 1], op=Alu.is_ge)
            miou = wk.tile([P, F], bf16, tag="miou")
            V.tensor_tensor(miou, iou[:, 0], iou[:, 1], op=Alu.max)
            resp = wk.tile([P, 2, F], bf16, tag="resp")
            V.tensor_tensor(resp[:, 0], ge, conf, op=Alu.mult)
            V.tensor_tensor(resp[:, 1], conf, resp[:, 0], op=Alu.subtract)

            # wh needs sqrt; nm = 1 - conf
            sq = wk.tile([P, 2, 4, F], bf16, tag="sq")
            A.activation(sq[:, 0], W, Act.Sqrt)
            A.activation(sq[:, 1], H, Act.Sqrt)
            nm = wk.tile([P, F], bf16, tag="nm")
            V.tensor_scalar(nm, conf, -1.0, 1.0, op0=Alu.mult, op1=Alu.add)

            # fix dx/dy box1 to slot-matched target, fill dw dh do nb
            V.tensor_tensor(dsq[:, 0, 1], X[:, 1], X[:, 3], op=Alu.subtract)
            V.tensor_tensor(dsq[:, 1, 1], Y[:, 1], Y[:, 3], op=Alu.subtract)
            V.tensor_tensor(dsq[:, 2], sq[:, 0, 0:2], sq[:, 0, 2:4],
                            op=Alu.subtract)
            V.tensor_tensor(dsq[:, 3], sq[:, 1, 0:2], sq[:, 1, 2:4],
                            op=Alu.subtract)
            V.tensor_tensor(dsq[:, 4], pc, _bc(miou, 2), op=Alu.subtract)
            V.tensor_tensor(dsq[:, 5], pc, _bc(nm, 2), op=Alu.mult)
            # mask by responsibility (resp^2 == resp), pipelined with the
            # square+accumulate passes
            V.tensor_tensor(dsq[:, 0:2], dsq[:, 0:2], _bc(resp, 2),
                            op=Alu.mult)
            A.activation(dsq[:, 0:2], dsq[:, 0:2], Act.Square,
                         accum_out=acc[:, 0:1])
            V.tensor_tensor(dsq[:, 2:4], dsq[:, 2:4], _bc(resp, 2),
                            op=Alu.mult)
            A.activation(dsq[:, 2:4], dsq[:, 2:4], Act.Square,
                         accum_out=acc[:, 1:2])
            mo = V.tensor_tensor(dsq[:, 4], dsq[:, 4], resp, op=Alu.mult)
            A.activation(dsq[:, 4], dsq[:, 4], Act.Square,
                         accum_out=acc[:, 2:3])
            A.activation(dsq[:, 5], dsq[:, 5], Act.Square,
                         accum_out=acc[:, 3:4])

            # class: mask by obj then square+accumulate (kept out of the
            # box chain's way with ordering-only edges)
            for q in range(NCQ):
                cq = conf[:, q * MQ:(q + 1) * MQ]
                mi = V.tensor_tensor(cv[q], cv[q], _bc(cq, 20), op=Alu.mult)
                add_dep_helper(mi.ins, mo.ins, sync=False,
                               reason="cls mask after box masks")
                A.activation(cv[q], cv[q], Act.Square,
                             accum_out=acc[:, 4 + q:5 + q])

            nc.sync.dma_start(out.ap(), acc)

    nc.compile()
    return nc


_NC_CACHE = None


def _get_nc():
    global _NC_CACHE
    if _NC_CACHE is None:
        _NC_CACHE = build_nc()
    return _NC_CACHE


# box plane order: (src, channel): X4, W4, Y4, H4, C3
_PLANES = [(0, 0), (0, 5), (1, 0), (1, 5),
           (0, 2), (0, 7), (1, 2), (1, 7),
           (0, 1), (0, 6), (1, 1), (1, 6),
           (0, 3), (0, 8), (1, 3), (1, 8),
           (0, 4), (0, 9), (1, 4)]


def shard_inputs(pred_tensor, target_tensor):
    """Full [16384,7,7,30] f32 -> per-core planar bf16 box + fp8 class."""
    p = np.asarray(pred_tensor, dtype=np.float32).reshape(NCORES, P, F, 30)
    t = np.asarray(target_tensor, dtype=np.float32).reshape(NCORES, P, F, 30)
    src = (p, t)

    box = np.empty((NCORES, P, 19, F), dtype=ml_dtypes.bfloat16)
    for i, (s, ch) in enumerate(_PLANES):
        box[:, :, i] = src[s][..., ch]

    pv = p[..., 10:30].astype(ml_dtypes.float8_e3m4)
    nv = (-t[..., 10:30]).astype(ml_dtypes.float8_e3m4)

    def v_pack(x):  # [NCORES, P, F, 20] -> [NCORES, NCQ, P, 20, MQ]
        y = x.transpose(0, 1, 3, 2).reshape(NCORES, P, 20, NCQ, MQ)
        return np.ascontiguousarray(y.transpose(0, 3, 1, 2, 4))

    cvp, cvn = v_pack(pv), v_pack(nv)
    return [{"box": box[c], "cvp": cvp[c], "cvn": cvn[c]}
            for c in range(NCORES)]


def combine(results):
    """Per-core acc_out [P, NACC] -> 5-tuple of loss scalars."""
    total = np.zeros(5, dtype=np.float64)
    for r in results:
        a = r["acc_out"].astype(np.float64).sum(axis=0)
        total[:4] += a[:4]
        total[4] += a[4:].sum()
    total /= BATCH
    return tuple(np.float32(v) for v in total)


def kernel(pred_tensor, target_tensor):
    nc = _get_nc()
    in_maps = shard_inputs(pred_tensor, target_tensor)
    res = run_bass_kernel_spmd(nc, in_maps, core_ids=list(range(NCORES)))
    return combine(res.results)


# revision 26
# speedup vs baseline: 2.1703x; 1.0271x over previous
"""YOLO-v1-style loss on 8 Trainium2 NeuronCores (Bass/Tile), v3.

Data-parallel over batch: each core gets 2048 of 16384 batch elements
(100,352 cells as 128 partitions x 784 cells); per-partition partial sums
for the 5 loss terms are combined on the host.

Layout: host repacks channels into per-channel planes ([P, plane, cells])
so DVE tensor_tensor ops run dense step-1 bf16 at 2x mode and
tensor_scalar ops at 4x. scalar_tensor_tensor (1x only) is avoided.

IoU via the overlap identity (no corner materialization):
  overlap_x = min(3.5*(pw+tw) - |px-tx|, 7*min(pw,tw)), clamped at 0.

Engine split:
  - DVE: box pipeline + responsibility masks + 8 class channels
  - ACT: sqrt, f32 conversions for reciprocal, square+accumulate passes
  - Pool: 12 class channels (cell-major tile: broadcast mask has its
    step-0 dim innermost - the only fast Q7 pattern) + SWDGE DMA descr gen
  - DMA CCE: class diff (p - t) computed inline: p streams in with an
    fp8->bf16 cast, host-negated t accumulates with cce add.

Explicit add_dep_helper edges order the CCE chain against its readers
(Tile's shadow tracking under-waits on multi-DMA read-modify-write tiles).

Self-contained: hardcodes all shapes; needs numpy + ml_dtypes + concourse.
"""

import numpy as np
import ml_dtypes

import concourse.bass as bass
import concourse.bacc as bacc
import concourse.tile as tile
import concourse.mybir as mybir
from concourse.bass_utils import run_bass_kernel_spmd
from bass_rust import add_dep_helper

DISABLE_CLS = False
DISABLE_CLS_G = False

f32 = mybir.dt.float32
bf16 = mybir.dt.bfloat16
f8e3 = mybir.dt.float8e3
Alu = mybir.AluOpType
Act = mybir.ActivationFunctionType

S = 7
BATCH = 16384
NCORES = 8
PER = BATCH // NCORES          # 2048 batch elems per core
P = 128                        # partitions
F = PER * S * S // P           # 784 cells per partition
NCQ = 4                        # class cell chunks
MQ = F // NCQ                  # 392
CHV = 8                        # class channels masked on DVE (channel-major)
CHG = 20 - CHV                 # class channels masked on Pool (cell-major)
HS = S / 2.0                   # 3.5
S2 = float(S * S)              # 49

NACC = 4 + NCQ


def _bc(x, r):
    """[P, ...] -> [P, r, ...]: broadcast (step-0) over a new outer dim."""
    return bass.AP(tensor=x.tensor, offset=x.offset,
                   ap=[x.ap[0], [0, r]] + list(x.ap[1:]))


def _bc_in(x, r):
    """[P, n] -> [P, n, r]: broadcast with the step-0 dim innermost."""
    return bass.AP(tensor=x.tensor, offset=x.offset,
                   ap=list(x.ap) + [[0, r]])


def build_nc():
    nc = bacc.Bacc("TRN2", target_bir_lowering=False, debug=False,
                   num_devices=NCORES)
    # box planes (bf16) [P, 19, F]:
    #   0-3  X4 = px0 px1 tx0 tx1      4-7  W4 = pw0 pw1 tw0 tw1
    #   8-11 Y4 = py0 py1 ty0 ty1     12-15 H4 = ph0 ph1 th0 th1
    #   16-18 C3 = pc0 pc1 conf
    box = nc.dram_tensor("box", [P, 19, F], bf16, kind="ExternalInput")
    # class streams (fp8 e3m4), negated target; V part channel-major,
    # G part cell-major.
    cvp = nc.dram_tensor("cvp", [NCQ, P, 20, MQ], f8e3, kind="ExternalInput")
    cvn = nc.dram_tensor("cvn", [NCQ, P, 20, MQ], f8e3, kind="ExternalInput")

    out = nc.dram_tensor("acc_out", [P, NACC], f32, kind="ExternalOutput")

    V = nc.vector
    A = nc.scalar
    G = nc.gpsimd

    with tile.TileContext(nc) as tc:
        with (
            tc.tile_pool(name="inp", bufs=1) as inp,
            tc.tile_pool(name="cls", bufs=4) as clsb,
            tc.tile_pool(name="wk", bufs=1) as wk,
            tc.tile_pool(name="one", bufs=1) as one,
        ):
            acc = one.tile([P, NACC], f32)
            V.memset(acc, 0.0)

            # ---- box DMAs first: V work starts as soon as X4 lands ----
            bxa = inp.tile([P, 8, F], bf16, tag="bxa")   # X4 W4
            nc.sync.dma_start(bxa[:, 0:4], box.ap()[:, 0:4])
            i_w4 = nc.sync.dma_start(bxa[:, 4:8], box.ap()[:, 4:8])
            bxb = inp.tile([P, 8, F], bf16, tag="bxb")   # Y4 H4
            nc.sync.dma_start(bxb[:, 0:4], box.ap()[:, 8:12])
            i_h4 = nc.sync.dma_start(bxb[:, 4:8], box.ap()[:, 12:16])
            bxc = inp.tile([P, 3, F], bf16, tag="bxc")   # C3
            i_c3 = nc.sync.dma_start(bxc, box.ap()[:, 16:19])
            box_gate = [i_w4, i_h4, i_c3, i_c3]

            # ---- class diff stream: SWDGE cast + CCE add (d = p - t) ----
            # CCE accumulate caps at 2048 elems/partition per DMA: the
            # cast moves all 20 channels, the accumulate goes in 2 halves.
            cv = []
            for q in range(NCQ):
                dv = clsb.tile([P, 20, MQ], bf16, tag="dv")
                ic = G.dma_start(dv, cvp.ap()[q])
                # throttle class stream behind the box DMAs so the box
                # pipeline's inputs land at full HBM rate first
                add_dep_helper(ic.ins, box_gate[q].ins,
                               reason="cls stream after box stream")
                G.dma_start(dv[:, 0:10], cvn.ap()[q][:, 0:10],
                            accum_op=Alu.add)
                G.dma_start(dv[:, 10:20], cvn.ap()[q][:, 10:20],
                            accum_op=Alu.add)
                cv.append(dv)

            X, W = bxa[:, 0:4], bxa[:, 4:8]
            Y, H = bxb[:, 0:4], bxb[:, 4:8]
            pc = bxc[:, 0:2]
            conf = bxc[:, 2]

            # squared-loss ingredients [dx(2) dy(2) dw(2) dh(2) do(2) nb(2)]
            dsq = wk.tile([P, 6, 2, F], bf16, tag="dsq")

            # dx_b = px_b - tx0 (slot-0 target; dsq[0,1] re-done per slot later)
            V.tensor_tensor(dsq[:, 0], X[:, 0:2], _bc(X[:, 2], 2),
                            op=Alu.subtract)
            V.tensor_tensor(dsq[:, 1], Y[:, 0:2], _bc(Y[:, 2], 2),
                            op=Alu.subtract)
            adx = wk.tile([P, 2, 2, F], bf16, tag="adx")
            A.activation(adx[:, 0], dsq[:, 0], Act.Abs)
            A.activation(adx[:, 1], dsq[:, 1], Act.Abs)

            # u = 3.5*(pw_b + tw0) - |dx|
            u = wk.tile([P, 2, 2, F], bf16, tag="u")
            V.tensor_tensor(u[:, 0], W[:, 0:2], _bc(W[:, 2], 2), op=Alu.add)
            V.tensor_tensor(u[:, 1], H[:, 0:2], _bc(H[:, 2], 2), op=Alu.add)
            V.tensor_scalar(u, u, HS, None, op0=Alu.mult)
            V.tensor_tensor(u, u, adx, op=Alu.subtract)

            # m7 = 7 * min(pw_b, tw0); overlap = relu(min(u, m7))
            m7 = wk.tile([P, 2, 2, F], bf16, tag="m7")
            V.tensor_tensor(m7[:, 0], W[:, 0:2], _bc(W[:, 2], 2), op=Alu.min)
            V.tensor_tensor(m7[:, 1], H[:, 0:2], _bc(H[:, 2], 2), op=Alu.min)
            V.tensor_scalar(m7, m7, float(S), None, op0=Alu.mult)
            whr = wk.tile([P, 2, 2, F], bf16, tag="whr")
            V.tensor_tensor(whr, u, m7, op=Alu.min)
            V.tensor_scalar(whr, whr, 0.0, None, op0=Alu.max)

            inter = wk.tile([P, 2, F], bf16, tag="inter")
            V.tensor_tensor(inter, whr[:, 0], whr[:, 1], op=Alu.mult)

            # areas (x S^2): [ap0 ap1 at]
            ar = wk.tile([P, 3, F], bf16, tag="ar")
            V.tensor_scalar(ar, W[:, 0:3], S2, None, op0=Alu.mult)
            V.tensor_tensor(ar, ar, H[:, 0:3], op=Alu.mult)
            den = wk.tile([P, 2, F], bf16, tag="den")
            V.tensor_tensor(den, ar[:, 0:2], _bc(ar[:, 2], 2), op=Alu.add)
            V.tensor_tensor(den, den, inter, op=Alu.subtract)
            den32 = wk.tile([P, 2, F], f32, tag="den32")
            V.tensor_copy(den32, den)
            rden = wk.tile([P, 2, F], f32, tag="rden")
            V.reciprocal_approx_fast(rden, den32)
            rden16 = wk.tile([P, 2, F], bf16, tag="rden16")
            V.tensor_copy(rden16, rden)
            iou = wk.tile([P, 2, F], bf16, tag="iou")
            i_iou = V.tensor_tensor(iou, inter, rden16, op=Alu.mult)

            # responsibility selection (argmax ties -> box0, like jnp)
            ge = wk.tile([P, F], bf16, tag="ge")
            V.tensor_tensor(ge, iou[:, 0], iou[:, 1], op=Alu.is_ge)
            miou = wk.tile([P, F], bf16, tag="miou")
            i_miou = V.tensor_tensor(miou, iou[:, 0], iou[:, 1], op=Alu.max)
            resp = wk.tile([P, 2, F], bf16, tag="resp")
            V.tensor_tensor(resp[:, 0], ge, conf, op=Alu.mult)
            V.tensor_tensor(resp[:, 1], conf, resp[:, 0], op=Alu.subtract)

            # wh needs sqrt; nm = 1 - conf
            sq = wk.tile([P, 2, 4, F], bf16, tag="sq")
            A.activation(sq[:, 0], W, Act.Sqrt)
            A.activation(sq[:, 1], H, Act.Sqrt)
            nm = wk.tile([P, F], bf16, tag="nm")
            V.tensor_scalar(nm, conf, -1.0, 1.0, op0=Alu.mult, op1=Alu.add)

            # fix dx/dy box1 to slot-matched target, fill dw dh do nb
            V.tensor_tensor(dsq[:, 0, 1], X[:, 1], X[:, 3], op=Alu.subtract)
            V.tensor_tensor(dsq[:, 1, 1], Y[:, 1], Y[:, 3], op=Alu.subtract)
            V.tensor_tensor(dsq[:, 2], sq[:, 0, 0:2], sq[:, 0, 2:4],
                            op=Alu.subtract)
            V.tensor_tensor(dsq[:, 3], sq[:, 1, 0:2], sq[:, 1, 2:4],
                            op=Alu.subtract)
            V.tensor_tensor(dsq[:, 4], pc, _bc(miou, 2), op=Alu.subtract)
            V.tensor_tensor(dsq[:, 5], pc, _bc(nm, 2), op=Alu.mult)
            # mask by responsibility (resp^2 == resp), pipelined with the
            # square+accumulate passes
            V.tensor_tensor(dsq[:, 0:2], dsq[:, 0:2], _bc(resp, 2),
                            op=Alu.mult)
            A.activation(dsq[:, 0:2], dsq[:, 0:2], Act.Square,
                         accum_out=acc[:, 0:1])
            V.tensor_tensor(dsq[:, 2:4], dsq[:, 2:4], _bc(resp, 2),
                            op=Alu.mult)
            A.activation(dsq[:, 2:4], dsq[:, 2:4], Act.Square,
                         accum_out=acc[:, 1:2])
            mo = V.tensor_tensor(dsq[:, 4], dsq[:, 4], resp, op=Alu.mult)
            A.activation(dsq[:, 4], dsq[:, 4], Act.Square,
                         accum_out=acc[:, 2:3])
            A.activation(dsq[:, 5], dsq[:, 5], Act.Square,
                         accum_out=acc[:, 3:4])

            # class: mask by obj then square+accumulate; stagger the masks
            # into the box chain at points where their DMA data has landed
            gates = [i_iou, i_miou, mo, mo]
            for q in range(NCQ):
                cq = conf[:, q * MQ:(q + 1) * MQ]
                mi = V.tensor_tensor(cv[q], cv[q], _bc(cq, 20), op=Alu.mult)
                add_dep_helper(mi.ins, gates[q].ins, sync=False,
                               reason="cls mask staggered into box chain")
                A.activation(cv[q], cv[q], Act.Square,
                             accum_out=acc[:, 4 + q:5 + q])

            nc.sync.dma_start(out.ap(), acc)

    nc.compile()
    return nc


_NC_CACHE = None


def _get_nc():
    global _NC_CACHE
    if _NC_CACHE is None:
        _NC_CACHE = build_nc()
    return _NC_CACHE


# box plane order: (src, channel): X4, W4, Y4, H4, C3
_PLANES = [(0, 0), (0, 5), (1, 0), (1, 5),
           (0, 2), (0, 7), (1, 2), (1, 7),
           (0, 1), (0, 6), (1, 1), (1, 6),
           (0, 3), (0, 8), (1, 3), (1, 8),
           (0, 4), (0, 9), (1, 4)]


def shard_inputs(pred_tensor, target_tensor):
    """Full [16384,7,7,30] f32 -> per-core planar bf16 box + fp8 class."""
    p = np.asarray(pred_tensor, dtype=np.float32).reshape(NCORES, P, F, 30)
    t = np.asarray(target_tensor, dtype=np.float32).reshape(NCORES, P, F, 30)
    src = (p, t)

    box = np.empty((NCORES, P, 19, F), dtype=ml_dtypes.bfloat16)
    for i, (s, ch) in enumerate(_PLANES):
        box[:, :, i] = src[s][..., ch]

    pv = p[..., 10:30].astype(ml_dtypes.float8_e3m4)
    nv = (-t[..., 10:30]).astype(ml_dtypes.float8_e3m4)

    def v_pack(x):  # [NCORES, P, F, 20] -> [NCORES, NCQ, P, 20, MQ]
        y = x.transpose(0, 1, 3, 2).reshape(NCORES, P, 20, NCQ, MQ)
        return np.ascontiguousarray(y.transpose(0, 3, 1, 2, 4))

    cvp, cvn = v_pack(pv), v_pack(nv)
    return [{"box": box[c], "cvp": cvp[c], "cvn": cvn[c]}
            for c in range(NCORES)]


def combine(results):
    """Per-core acc_out [P, NACC] -> 5-tuple of loss scalars."""
    total = np.zeros(5, dtype=np.float64)
    for r in results:
        a = r["acc_out"].astype(np.float64).sum(axis=0)
        total[:4] += a[:4]
        total[4] += a[4:].sum()
    total /= BATCH
    return tuple(np.float32(v) for v in total)


def kernel(pred_tensor, target_tensor):
    nc = _get_nc()
    in_maps = shard_inputs(pred_tensor, target_tensor)
    res = run_bass_kernel_spmd(nc, in_maps, core_ids=list(range(NCORES)))
    return combine(res.results)


# revision 27
# speedup vs baseline: 2.2472x; 1.0354x over previous
"""YOLO-v1-style loss on 8 Trainium2 NeuronCores (Bass/Tile), v3.

Data-parallel over batch: each core gets 2048 of 16384 batch elements
(100,352 cells as 128 partitions x 784 cells); per-partition partial sums
for the 5 loss terms are combined on the host.

Layout: host repacks channels into per-channel planes ([P, plane, cells])
so DVE tensor_tensor ops run dense step-1 bf16 at 2x mode and
tensor_scalar ops at 4x. scalar_tensor_tensor (1x only) is avoided.

IoU via the overlap identity (no corner materialization):
  overlap_x = min(3.5*(pw+tw) - |px-tx|, 7*min(pw,tw)), clamped at 0.

Engine split:
  - DVE: box pipeline + responsibility masks + 8 class channels
  - ACT: sqrt, f32 conversions for reciprocal, square+accumulate passes
  - Pool: 12 class channels (cell-major tile: broadcast mask has its
    step-0 dim innermost - the only fast Q7 pattern) + SWDGE DMA descr gen
  - DMA CCE: class diff (p - t) computed inline: p streams in with an
    fp8->bf16 cast, host-negated t accumulates with cce add.

Explicit add_dep_helper edges order the CCE chain against its readers
(Tile's shadow tracking under-waits on multi-DMA read-modify-write tiles).

Self-contained: hardcodes all shapes; needs numpy + ml_dtypes + concourse.
"""

import numpy as np
import ml_dtypes

import concourse.bass as bass
import concourse.bacc as bacc
import concourse.tile as tile
import concourse.mybir as mybir
from concourse.bass_utils import run_bass_kernel_spmd
from bass_rust import add_dep_helper

DISABLE_CLS = False
DISABLE_CLS_G = False

f32 = mybir.dt.float32
bf16 = mybir.dt.bfloat16
f8e3 = mybir.dt.float8e3
Alu = mybir.AluOpType
Act = mybir.ActivationFunctionType

S = 7
BATCH = 16384
NCORES = 8
PER = BATCH // NCORES          # 2048 batch elems per core
P = 128                        # partitions
F = PER * S * S // P           # 784 cells per partition
NCQ = 4                        # class cell chunks
MQ = F // NCQ                  # 392
CHV = 8                        # class channels masked on DVE (channel-major)
CHG = 20 - CHV                 # class channels masked on Pool (cell-major)
HS = S / 2.0                   # 3.5
S2 = float(S * S)              # 49

NACC = 4 + NCQ


def _bc(x, r):
    """[P, ...] -> [P, r, ...]: broadcast (step-0) over a new outer dim."""
    return bass.AP(tensor=x.tensor, offset=x.offset,
                   ap=[x.ap[0], [0, r]] + list(x.ap[1:]))


def _bc_in(x, r):
    """[P, n] -> [P, n, r]: broadcast with the step-0 dim innermost."""
    return bass.AP(tensor=x.tensor, offset=x.offset,
                   ap=list(x.ap) + [[0, r]])


def build_nc():
    nc = bacc.Bacc("TRN2", target_bir_lowering=False, debug=False,
                   num_devices=NCORES)
    # box planes (bf16) [P, 19, F]:
    #   0-3  X4 = px0 px1 tx0 tx1      4-7  W4 = pw0 pw1 tw0 tw1
    #   8-11 Y4 = py0 py1 ty0 ty1     12-15 H4 = ph0 ph1 th0 th1
    #   16-18 C3 = pc0 pc1 conf
    box = nc.dram_tensor("box", [P, 19, F], bf16, kind="ExternalInput")
    # class streams (fp8 e3m4), negated target; V part channel-major,
    # G part cell-major.
    cvp = nc.dram_tensor("cvp", [NCQ, P, 20, MQ], f8e3, kind="ExternalInput")
    cvn = nc.dram_tensor("cvn", [NCQ, P, 20, MQ], f8e3, kind="ExternalInput")

    out = nc.dram_tensor("acc_out", [P, NACC], f32, kind="ExternalOutput")

    V = nc.vector
    A = nc.scalar
    G = nc.gpsimd

    with tile.TileContext(nc) as tc:
        with (
            tc.tile_pool(name="inp", bufs=1) as inp,
            tc.tile_pool(name="cls", bufs=4) as clsb,
            tc.tile_pool(name="wk", bufs=1) as wk,
            tc.tile_pool(name="one", bufs=1) as one,
        ):
            acc = one.tile([P, NACC], f32)
            V.memset(acc, 0.0)

            # ---- box DMAs first: V work starts as soon as X4 lands ----
            bxa = inp.tile([P, 8, F], bf16, tag="bxa")   # X4 W4
            nc.sync.dma_start(bxa[:, 0:4], box.ap()[:, 0:4])
            i_w4 = nc.sync.dma_start(bxa[:, 4:8], box.ap()[:, 4:8])
            bxb = inp.tile([P, 8, F], bf16, tag="bxb")   # Y4 H4
            nc.sync.dma_start(bxb[:, 0:4], box.ap()[:, 8:12])
            i_h4 = nc.sync.dma_start(bxb[:, 4:8], box.ap()[:, 12:16])
            bxc = inp.tile([P, 3, F], bf16, tag="bxc")   # C3
            i_c3 = nc.sync.dma_start(bxc, box.ap()[:, 16:19])
            box_gate = [i_w4, i_h4, i_c3, i_c3]

            # ---- class diff stream: SWDGE cast + CCE add (d = p - t) ----
            # CCE accumulate caps at 2048 elems/partition per DMA: the
            # cast moves all 20 channels, the accumulate goes in 2 halves.
            cv = []
            for q in range(NCQ):
                dv = clsb.tile([P, 20, MQ], bf16, tag="dv")
                ic = G.dma_start(dv, cvp.ap()[q])
                # throttle class stream behind the box DMAs so the box
                # pipeline's inputs land at full HBM rate first
                add_dep_helper(ic.ins, box_gate[q].ins,
                               reason="cls stream after box stream")
                G.dma_start(dv[:, 0:10], cvn.ap()[q][:, 0:10],
                            accum_op=Alu.add)
                G.dma_start(dv[:, 10:20], cvn.ap()[q][:, 10:20],
                            accum_op=Alu.add)
                cv.append(dv)

            X, W = bxa[:, 0:4], bxa[:, 4:8]
            Y, H = bxb[:, 0:4], bxb[:, 4:8]
            pc = bxc[:, 0:2]
            conf = bxc[:, 2]

            # squared-loss ingredients [dx(2) dy(2) dw(2) dh(2) do(2) nb(2)]
            dsq = wk.tile([P, 6, 2, F], bf16, tag="dsq")

            # dx_b = px_b - tx0 (slot-0 target; dsq[0,1] re-done per slot later)
            V.tensor_tensor(dsq[:, 0], X[:, 0:2], _bc(X[:, 2], 2),
                            op=Alu.subtract)
            V.tensor_tensor(dsq[:, 1], Y[:, 0:2], _bc(Y[:, 2], 2),
                            op=Alu.subtract)
            adx = wk.tile([P, 2, 2, F], bf16, tag="adx")
            A.activation(adx[:, 0], dsq[:, 0], Act.Abs)
            A.activation(adx[:, 1], dsq[:, 1], Act.Abs)

            # u = 3.5*(pw_b + tw0) - |dx|
            u = wk.tile([P, 2, 2, F], bf16, tag="u")
            V.tensor_tensor(u[:, 0], W[:, 0:2], _bc(W[:, 2], 2), op=Alu.add)
            V.tensor_tensor(u[:, 1], H[:, 0:2], _bc(H[:, 2], 2), op=Alu.add)
            V.tensor_scalar(u, u, HS, None, op0=Alu.mult)
            V.tensor_tensor(u, u, adx, op=Alu.subtract)

            # m7 = 7 * min(pw_b, tw0); overlap = relu(min(u, m7))
            m7 = wk.tile([P, 2, 2, F], bf16, tag="m7")
            V.tensor_tensor(m7[:, 0], W[:, 0:2], _bc(W[:, 2], 2), op=Alu.min)
            V.tensor_tensor(m7[:, 1], H[:, 0:2], _bc(H[:, 2], 2), op=Alu.min)
            V.tensor_scalar(m7, m7, float(S), None, op0=Alu.mult)
            whr = wk.tile([P, 2, 2, F], bf16, tag="whr")
            V.tensor_tensor(whr, u, m7, op=Alu.min)
            V.tensor_scalar(whr, whr, 0.0, None, op0=Alu.max)

            inter = wk.tile([P, 2, F], bf16, tag="inter")
            V.tensor_tensor(inter, whr[:, 0], whr[:, 1], op=Alu.mult)

            # areas (x S^2): [ap0 ap1 at]
            ar = wk.tile([P, 3, F], bf16, tag="ar")
            V.tensor_scalar(ar, W[:, 0:3], S2, None, op0=Alu.mult)
            V.tensor_tensor(ar, ar, H[:, 0:3], op=Alu.mult)
            den = wk.tile([P, 2, F], bf16, tag="den")
            V.tensor_tensor(den, ar[:, 0:2], _bc(ar[:, 2], 2), op=Alu.add)
            V.tensor_tensor(den, den, inter, op=Alu.subtract)
            den32 = wk.tile([P, 2, F], f32, tag="den32")
            V.tensor_copy(den32, den)
            rden = wk.tile([P, 2, F], f32, tag="rden")
            V.reciprocal_approx_fast(rden, den32)
            rden16 = wk.tile([P, 2, F], bf16, tag="rden16")
            V.tensor_copy(rden16, rden)
            iou = wk.tile([P, 2, F], bf16, tag="iou")
            i_iou = V.tensor_tensor(iou, inter, rden16, op=Alu.mult)

            # responsibility selection (argmax ties -> box0, like jnp)
            ge = wk.tile([P, F], bf16, tag="ge")
            V.tensor_tensor(ge, iou[:, 0], iou[:, 1], op=Alu.is_ge)
            miou = wk.tile([P, F], bf16, tag="miou")
            i_miou = V.tensor_tensor(miou, iou[:, 0], iou[:, 1], op=Alu.max)
            resp = wk.tile([P, 2, F], bf16, tag="resp")
            V.tensor_tensor(resp[:, 0], ge, conf, op=Alu.mult)
            V.tensor_tensor(resp[:, 1], conf, resp[:, 0], op=Alu.subtract)

            # wh needs sqrt; nm = 1 - conf
            sq = wk.tile([P, 2, 4, F], bf16, tag="sq")
            A.activation(sq[:, 0], W, Act.Sqrt)
            A.activation(sq[:, 1], H, Act.Sqrt)
            nm = wk.tile([P, F], bf16, tag="nm")
            V.tensor_scalar(nm, conf, -1.0, 1.0, op0=Alu.mult, op1=Alu.add)

            # fix dx/dy box1 to slot-matched target, fill dw dh do nb
            V.tensor_tensor(dsq[:, 0, 1], X[:, 1], X[:, 3], op=Alu.subtract)
            V.tensor_tensor(dsq[:, 1, 1], Y[:, 1], Y[:, 3], op=Alu.subtract)
            V.tensor_tensor(dsq[:, 2], sq[:, 0, 0:2], sq[:, 0, 2:4],
                            op=Alu.subtract)
            V.tensor_tensor(dsq[:, 3], sq[:, 1, 0:2], sq[:, 1, 2:4],
                            op=Alu.subtract)
            V.tensor_tensor(dsq[:, 4], pc, _bc(miou, 2), op=Alu.subtract)
            V.tensor_tensor(dsq[:, 5], pc, _bc(nm, 2), op=Alu.mult)
            # mask by responsibility (resp^2 == resp), pipelined with the
            # square+accumulate passes
            V.tensor_tensor(dsq[:, 0:2], dsq[:, 0:2], _bc(resp, 2),
                            op=Alu.mult)
            A.activation(dsq[:, 0:2], dsq[:, 0:2], Act.Square,
                         accum_out=acc[:, 0:1])
            V.tensor_tensor(dsq[:, 2:4], dsq[:, 2:4], _bc(resp, 2),
                            op=Alu.mult)
            A.activation(dsq[:, 2:4], dsq[:, 2:4], Act.Square,
                         accum_out=acc[:, 1:2])
            mo = V.tensor_tensor(dsq[:, 4], dsq[:, 4], resp, op=Alu.mult)
            A.activation(dsq[:, 4], dsq[:, 4], Act.Square,
                         accum_out=acc[:, 2:3])
            A.activation(dsq[:, 5], dsq[:, 5], Act.Square,
                         accum_out=acc[:, 3:4])

            # class: mask by obj then square+accumulate; stagger the masks
            # into the box chain at points where their DMA data has landed.
            # The last quarter's reduction runs on DVE (stt) to shorten the
            # ACT tail.
            gates = [i_iou, i_miou, i_miou, mo]
            trash = wk.tile([P, 20, MQ], bf16, tag="trash")
            for q in range(NCQ):
                cq = conf[:, q * MQ:(q + 1) * MQ]
                mi = V.tensor_tensor(cv[q], cv[q], _bc(cq, 20), op=Alu.mult)
                add_dep_helper(mi.ins, gates[q].ins, sync=False,
                               reason="cls mask staggered into box chain")
                if q == NCQ - 1:
                    V.scalar_tensor_tensor(trash, cv[q], 0.0, cv[q],
                                           op0=Alu.bypass, op1=Alu.mult,
                                           accum_out=acc[:, 4 + q:5 + q])
                else:
                    A.activation(cv[q], cv[q], Act.Square,
                                 accum_out=acc[:, 4 + q:5 + q])

            nc.sync.dma_start(out.ap(), acc)

    nc.compile()
    return nc


_NC_CACHE = None


def _get_nc():
    global _NC_CACHE
    if _NC_CACHE is None:
        _NC_CACHE = build_nc()
    return _NC_CACHE


# box plane order: (src, channel): X4, W4, Y4, H4, C3
_PLANES = [(0, 0), (0, 5), (1, 0), (1, 5),
           (0, 2), (0, 7), (1, 2), (1, 7),
           (0, 1), (0, 6), (1, 1), (1, 6),
           (0, 3), (0, 8), (1, 3), (1, 8),
           (0, 4), (0, 9), (1, 4)]


def shard_inputs(pred_tensor, target_tensor):
    """Full [16384,7,7,30] f32 -> per-core planar bf16 box + fp8 class."""
    p = np.asarray(pred_tensor, dtype=np.float32).reshape(NCORES, P, F, 30)
    t = np.asarray(target_tensor, dtype=np.float32).reshape(NCORES, P, F, 30)
    src = (p, t)

    box = np.empty((NCORES, P, 19, F), dtype=ml_dtypes.bfloat16)
    for i, (s, ch) in enumerate(_PLANES):
        box[:, :, i] = src[s][..., ch]

    pv = p[..., 10:30].astype(ml_dtypes.float8_e3m4)
    nv = (-t[..., 10:30]).astype(ml_dtypes.float8_e3m4)

    def v_pack(x):  # [NCORES, P, F, 20] -> [NCORES, NCQ, P, 20, MQ]
        y = x.transpose(0, 1, 3, 2).reshape(NCORES, P, 20, NCQ, MQ)
        return np.ascontiguousarray(y.transpose(0, 3, 1, 2, 4))

    cvp, cvn = v_pack(pv), v_pack(nv)
    return [{"box": box[c], "cvp": cvp[c], "cvn": cvn[c]}
            for c in range(NCORES)]


def combine(results):
    """Per-core acc_out [P, NACC] -> 5-tuple of loss scalars."""
    total = np.zeros(5, dtype=np.float64)
    for r in results:
        a = r["acc_out"].astype(np.float64).sum(axis=0)
        total[:4] += a[:4]
        total[4] += a[4:].sum()
    total /= BATCH
    return tuple(np.float32(v) for v in total)


def kernel(pred_tensor, target_tensor):
    nc = _get_nc()
    in_maps = shard_inputs(pred_tensor, target_tensor)
    res = run_bass_kernel_spmd(nc, in_maps, core_ids=list(range(NCORES)))
    return combine(res.results)


# revision 28
# speedup vs baseline: 2.2765x; 1.0131x over previous
"""YOLO-v1-style loss on 8 Trainium2 NeuronCores (Bass/Tile), v3.

Data-parallel over batch: each core gets 2048 of 16384 batch elements
(100,352 cells as 128 partitions x 784 cells); per-partition partial sums
for the 5 loss terms are combined on the host.

Layout: host repacks channels into per-channel planes ([P, plane, cells])
so DVE tensor_tensor ops run dense step-1 bf16 at 2x mode and
tensor_scalar ops at 4x. scalar_tensor_tensor (1x only) is avoided.

IoU via the overlap identity (no corner materialization):
  overlap_x = min(3.5*(pw+tw) - |px-tx|, 7*min(pw,tw)), clamped at 0.

Engine split:
  - DVE: box pipeline (2x-mode bf16 tensor_tensor) + obj/resp masks
  - ACT: sqrt/abs, square+accumulate reduction passes
  - Pool: SWDGE descriptor generation only (its tensor ops are slow and
    fight DVE for SBUF ports)
  - DMA CCE: class diff (p - t) computed inline in the DMA engines:
    p streams in with an fp8->bf16 cast, host-negated t accumulates with
    cce add. CCE accumulates silently corrupt beyond 2048 elements per
    partition per DMA, so the accumulate is issued in <=1960-element calls.

add_dep_helper sync=False edges steer the Tile list scheduler: the class
masks slot into the box chain where their DMA data has landed, and the
class stream is throttled behind the box stream for HBM priority.

Self-contained: hardcodes all shapes; needs numpy + ml_dtypes + concourse.
"""

import numpy as np
import ml_dtypes

import concourse.bass as bass
import concourse.bacc as bacc
import concourse.tile as tile
import concourse.mybir as mybir
from concourse.bass_utils import run_bass_kernel_spmd
from bass_rust import add_dep_helper

f32 = mybir.dt.float32
bf16 = mybir.dt.bfloat16
f8e3 = mybir.dt.float8e3
Alu = mybir.AluOpType
Act = mybir.ActivationFunctionType

S = 7
BATCH = 16384
NCORES = 8
PER = BATCH // NCORES          # 2048 batch elems per core
P = 128                        # partitions
F = PER * S * S // P           # 784 cells per partition
NCQ = 4                        # class cell chunks
MQ = F // NCQ                  # 392
HS = S / 2.0                   # 3.5
S2 = float(S * S)              # 49

NACC = 4 + NCQ


def _bc(x, r):
    """[P, ...] -> [P, r, ...]: broadcast (step-0) over a new outer dim."""
    return bass.AP(tensor=x.tensor, offset=x.offset,
                   ap=[x.ap[0], [0, r]] + list(x.ap[1:]))


def build_nc():
    nc = bacc.Bacc("TRN2", target_bir_lowering=False, debug=False,
                   num_devices=NCORES)
    # box planes (bf16) [P, 19, F]:
    #   0-3  X4 = px0 px1 tx0 tx1      4-7  W4 = pw0 pw1 tw0 tw1
    #   8-11 Y4 = py0 py1 ty0 ty1     12-15 H4 = ph0 ph1 th0 th1
    #   16-18 C3 = pc0 pc1 conf
    box = nc.dram_tensor("box", [P, 19, F], bf16, kind="ExternalInput")
    # class streams (fp8 e3m4), negated target; V part channel-major,
    # G part cell-major.
    cvp = nc.dram_tensor("cvp", [NCQ, P, 20, MQ], f8e3, kind="ExternalInput")
    cvn = nc.dram_tensor("cvn", [NCQ, P, 20, MQ], f8e3, kind="ExternalInput")

    out = nc.dram_tensor("acc_out", [P, NACC], f32, kind="ExternalOutput")

    V = nc.vector
    A = nc.scalar
    G = nc.gpsimd

    with tile.TileContext(nc) as tc:
        with (
            tc.tile_pool(name="inp", bufs=1) as inp,
            tc.tile_pool(name="cls", bufs=4) as clsb,
            tc.tile_pool(name="wk", bufs=1) as wk,
            tc.tile_pool(name="one", bufs=1) as one,
        ):
            acc = one.tile([P, NACC], f32)
            V.memset(acc, 0.0)

            # ---- box DMAs first: V work starts as soon as X4 lands ----
            bxa = inp.tile([P, 8, F], bf16, tag="bxa")   # X4 W4
            nc.sync.dma_start(bxa[:, 0:4], box.ap()[:, 0:4])
            i_w4 = nc.sync.dma_start(bxa[:, 4:8], box.ap()[:, 4:8])
            bxb = inp.tile([P, 8, F], bf16, tag="bxb")   # Y4 H4
            nc.sync.dma_start(bxb[:, 0:4], box.ap()[:, 8:12])
            i_h4 = nc.sync.dma_start(bxb[:, 4:8], box.ap()[:, 12:16])
            bxc = inp.tile([P, 3, F], bf16, tag="bxc")   # C3
            i_c3 = nc.sync.dma_start(bxc, box.ap()[:, 16:19])
            box_gate = [i_w4, i_h4, i_c3, i_c3]

            # ---- class diff stream: SWDGE cast + CCE add (d = p - t) ----
            # CCE accumulate caps at 2048 elems/partition per DMA: the
            # cast moves all 20 channels, the accumulate goes in 2 halves.
            cv = []
            for q in range(NCQ):
                dv = clsb.tile([P, 20, MQ], bf16, tag="dv")
                ic = G.dma_start(dv, cvp.ap()[q])
                # throttle class stream behind the box DMAs so the box
                # pipeline's inputs land at full HBM rate first
                add_dep_helper(ic.ins, box_gate[q].ins,
                               reason="cls stream after box stream")
                G.dma_start(dv[:, 0:10], cvn.ap()[q][:, 0:10],
                            accum_op=Alu.add)
                G.dma_start(dv[:, 10:20], cvn.ap()[q][:, 10:20],
                            accum_op=Alu.add)
                cv.append(dv)

            X, W = bxa[:, 0:4], bxa[:, 4:8]
            Y, H = bxb[:, 0:4], bxb[:, 4:8]
            pc = bxc[:, 0:2]
            conf = bxc[:, 2]

            # squared-loss ingredients [dx(2) dy(2) dw(2) dh(2) do(2) nb(2)]
            dsq = wk.tile([P, 6, 2, F], bf16, tag="dsq")

            # dx_b = px_b - tx0 (slot-0 target; dsq[0,1] re-done per slot later)
            V.tensor_tensor(dsq[:, 0], X[:, 0:2], _bc(X[:, 2], 2),
                            op=Alu.subtract)
            V.tensor_tensor(dsq[:, 1], Y[:, 0:2], _bc(Y[:, 2], 2),
                            op=Alu.subtract)
            adx = wk.tile([P, 2, 2, F], bf16, tag="adx")
            A.activation(adx[:, 0], dsq[:, 0], Act.Abs)
            A.activation(adx[:, 1], dsq[:, 1], Act.Abs)

            # u = 3.5*(pw_b + tw0) - |dx|
            u = wk.tile([P, 2, 2, F], bf16, tag="u")
            V.tensor_tensor(u[:, 0], W[:, 0:2], _bc(W[:, 2], 2), op=Alu.add)
            V.tensor_tensor(u[:, 1], H[:, 0:2], _bc(H[:, 2], 2), op=Alu.add)
            V.tensor_scalar(u, u, HS, None, op0=Alu.mult)
            V.tensor_tensor(u, u, adx, op=Alu.subtract)

            # m7 = 7 * min(pw_b, tw0); overlap = relu(min(u, m7))
            m7 = wk.tile([P, 2, 2, F], bf16, tag="m7")
            V.tensor_tensor(m7[:, 0], W[:, 0:2], _bc(W[:, 2], 2), op=Alu.min)
            V.tensor_tensor(m7[:, 1], H[:, 0:2], _bc(H[:, 2], 2), op=Alu.min)
            V.tensor_scalar(m7, m7, float(S), None, op0=Alu.mult)
            whr = wk.tile([P, 2, 2, F], bf16, tag="whr")
            V.tensor_tensor(whr, u, m7, op=Alu.min)
            V.tensor_scalar(whr, whr, 0.0, None, op0=Alu.max)

            inter = wk.tile([P, 2, F], bf16, tag="inter")
            V.tensor_tensor(inter, whr[:, 0], whr[:, 1], op=Alu.mult)

            # areas (x S^2): [ap0 ap1 at]
            ar = wk.tile([P, 3, F], bf16, tag="ar")
            V.tensor_scalar(ar, W[:, 0:3], S2, None, op0=Alu.mult)
            V.tensor_tensor(ar, ar, H[:, 0:3], op=Alu.mult)
            den = wk.tile([P, 2, F], bf16, tag="den")
            V.tensor_tensor(den, ar[:, 0:2], _bc(ar[:, 2], 2), op=Alu.add)
            V.tensor_tensor(den, den, inter, op=Alu.subtract)
            den32 = wk.tile([P, 2, F], f32, tag="den32")
            V.tensor_copy(den32, den)
            rden = wk.tile([P, 2, F], f32, tag="rden")
            V.reciprocal_approx_fast(rden, den32)
            rden16 = wk.tile([P, 2, F], bf16, tag="rden16")
            V.tensor_copy(rden16, rden)
            iou = wk.tile([P, 2, F], bf16, tag="iou")
            i_iou = V.tensor_tensor(iou, inter, rden16, op=Alu.mult)

            # responsibility selection (argmax ties -> box0, like jnp)
            ge = wk.tile([P, F], bf16, tag="ge")
            V.tensor_tensor(ge, iou[:, 0], iou[:, 1], op=Alu.is_ge)
            miou = wk.tile([P, F], bf16, tag="miou")
            i_miou = V.tensor_tensor(miou, iou[:, 0], iou[:, 1], op=Alu.max)
            resp = wk.tile([P, 2, F], bf16, tag="resp")
            V.tensor_tensor(resp[:, 0], ge, conf, op=Alu.mult)
            V.tensor_tensor(resp[:, 1], conf, resp[:, 0], op=Alu.subtract)

            # wh needs sqrt; nm = 1 - conf
            sq = wk.tile([P, 2, 4, F], bf16, tag="sq")
            A.activation(sq[:, 0], W, Act.Sqrt)
            A.activation(sq[:, 1], H, Act.Sqrt)
            nm = wk.tile([P, F], bf16, tag="nm")
            V.tensor_scalar(nm, conf, -1.0, 1.0, op0=Alu.mult, op1=Alu.add)

            # fix dx/dy box1 to slot-matched target, fill dw dh do nb
            V.tensor_tensor(dsq[:, 0, 1], X[:, 1], X[:, 3], op=Alu.subtract)
            V.tensor_tensor(dsq[:, 1, 1], Y[:, 1], Y[:, 3], op=Alu.subtract)
            V.tensor_tensor(dsq[:, 2], sq[:, 0, 0:2], sq[:, 0, 2:4],
                            op=Alu.subtract)
            V.tensor_tensor(dsq[:, 3], sq[:, 1, 0:2], sq[:, 1, 2:4],
                            op=Alu.subtract)
            V.tensor_tensor(dsq[:, 4], pc, _bc(miou, 2), op=Alu.subtract)
            V.tensor_tensor(dsq[:, 5], pc, _bc(nm, 2), op=Alu.mult)
            # mask by responsibility (resp^2 == resp), pipelined with the
            # square+accumulate passes
            V.tensor_tensor(dsq[:, 0:2], dsq[:, 0:2], _bc(resp, 2),
                            op=Alu.mult)
            A.activation(dsq[:, 0:2], dsq[:, 0:2], Act.Square,
                         accum_out=acc[:, 0:1])
            V.tensor_tensor(dsq[:, 2:4], dsq[:, 2:4], _bc(resp, 2),
                            op=Alu.mult)
            A.activation(dsq[:, 2:4], dsq[:, 2:4], Act.Square,
                         accum_out=acc[:, 1:2])
            mo = V.tensor_tensor(dsq[:, 4], dsq[:, 4], resp, op=Alu.mult)
            A.activation(dsq[:, 4], dsq[:, 4], Act.Square,
                         accum_out=acc[:, 2:3])
            A.activation(dsq[:, 5], dsq[:, 5], Act.Square,
                         accum_out=acc[:, 3:4])

            # class: mask by obj then square+accumulate; stagger the masks
            # into the box chain at points where their DMA data has landed.
            # The last quarter's reduction runs on DVE (stt) to shorten the
            # ACT tail.
            gates = [i_iou, i_miou, i_miou, mo]
            trash = wk.tile([P, 20, MQ], bf16, tag="trash")
            for q in range(NCQ):
                cq = conf[:, q * MQ:(q + 1) * MQ]
                mi = V.tensor_tensor(cv[q], cv[q], _bc(cq, 20), op=Alu.mult)
                add_dep_helper(mi.ins, gates[q].ins, sync=False,
                               reason="cls mask staggered into box chain")
                if q == NCQ - 1:
                    V.scalar_tensor_tensor(trash, cv[q], 0.0, cv[q],
                                           op0=Alu.bypass, op1=Alu.mult,
                                           accum_out=acc[:, 4 + q:5 + q])
                else:
                    A.activation(cv[q], cv[q], Act.Square,
                                 accum_out=acc[:, 4 + q:5 + q])

            nc.sync.dma_start(out.ap(), acc)

    nc.compile()
    return nc


_NC_CACHE = None


def _get_nc():
    global _NC_CACHE
    if _NC_CACHE is None:
        _NC_CACHE = build_nc()
    return _NC_CACHE


# box plane order: (src, channel): X4, W4, Y4, H4, C3
_PLANES = [(0, 0), (0, 5), (1, 0), (1, 5),
           (0, 2), (0, 7), (1, 2), (1, 7),
           (0, 1), (0, 6), (1, 1), (1, 6),
           (0, 3), (0, 8), (1, 3), (1, 8),
           (0, 4), (0, 9), (1, 4)]


def shard_inputs(pred_tensor, target_tensor):
    """Full [16384,7,7,30] f32 -> per-core planar bf16 box + fp8 class."""
    p = np.asarray(pred_tensor, dtype=np.float32).reshape(NCORES, P, F, 30)
    t = np.asarray(target_tensor, dtype=np.float32).reshape(NCORES, P, F, 30)
    src = (p, t)

    box = np.empty((NCORES, P, 19, F), dtype=ml_dtypes.bfloat16)
    for i, (s, ch) in enumerate(_PLANES):
        box[:, :, i] = src[s][..., ch]

    pv = p[..., 10:30].astype(ml_dtypes.float8_e3m4)
    nv = (-t[..., 10:30]).astype(ml_dtypes.float8_e3m4)

    def v_pack(x):  # [NCORES, P, F, 20] -> [NCORES, NCQ, P, 20, MQ]
        y = x.transpose(0, 1, 3, 2).reshape(NCORES, P, 20, NCQ, MQ)
        return np.ascontiguousarray(y.transpose(0, 3, 1, 2, 4))

    cvp, cvn = v_pack(pv), v_pack(nv)
    return [{"box": box[c], "cvp": cvp[c], "cvn": cvn[c]}
            for c in range(NCORES)]


def combine(results):
    """Per-core acc_out [P, NACC] -> 5-tuple of loss scalars."""
    total = np.zeros(5, dtype=np.float64)
    for r in results:
        a = r["acc_out"].astype(np.float64).sum(axis=0)
        total[:4] += a[:4]
        total[4] += a[4:].sum()
    total /= BATCH
    return tuple(np.float32(v) for v in total)


def kernel(pred_tensor, target_tensor):
    nc = _get_nc()
    in_maps = shard_inputs(pred_tensor, target_tensor)
    res = run_bass_kernel_spmd(nc, in_maps, core_ids=list(range(NCORES)))
    return combine(res.results)
